# revision 1
# baseline (speedup 1.0000x reference)
"""Trainium2 Bass kernel for nn_Attention_Sep (sparse attention, B=16 N=1025 C=768 H=12 nb=4).

Data-parallel over batch (2 per core, 8 cores). Per core, fp32r matmuls in a
transposed [feature, token] layout:
  A) Q^T/K^T feature-tiles = W_qkv.T @ x^T; V natural (x^T as lhsT) stored in an
     augmented head-pair group layout [A(64)|ones|pad31|B(64)] (160 cols/group)
     so the PV matmul later produces softmax denominators for free.
  B) cls attention over all 1025 tokens: block-diagonal q0 lhsT -> [12, tok]
     logits, row softmax (exp+accum on ACT), PE-transposed weights, V contraction,
     + residual -> A^T column 0.
  C) recompute k,v of the updated cls token (row matmuls + tiny transposes).
  D) 4 branches x 6 head-pairs: S^T row-pair-packed (2 heads concurrently in
     different PSUM banks), exp with folded 1/8 scale, PV with denominators at
     rows 64 (head A, group[0:65]) / 32 (head B, group[32:160]); reciprocal +
     DMA partition-broadcast + DVE multiply -> normalized A^T in place.
  E) out = A^T.T @ W_proj + b_proj -> DRAM natural layout.
"""
import sys, types
import numpy as np


def _ensure_ntff_hook():
    try:
        import antenv
        if "antenv.axon_hooks" in sys.modules:
            return
        from trn_agent_boot.trn_boot import _ntff_profile_via_ctypes
        mod = types.ModuleType("antenv.axon_hooks")
        mod._hook = None
        mod.set_axon_ntff_profile_hook = lambda h: setattr(mod, "_hook", h)
        mod.get_axon_ntff_profile_hook = lambda: mod._hook
        sys.modules["antenv.axon_hooks"] = mod
        antenv.axon_hooks = mod
        mod.set_axon_ntff_profile_hook(_ntff_profile_via_ctypes('/opt/axon/libaxon_pjrt.so'))
    except Exception:
        pass


_NC_CACHE = {}


def build_program():
    if "nc" in _NC_CACHE:
        return _NC_CACHE["nc"]
    import concourse.bass as bass
    import concourse.mybir as mybir
    import concourse.tile as tile
    from concourse import bacc
    from concourse.masks import make_identity

    F32, F32R = mybir.dt.float32, mybir.dt.float32r
    AF = mybir.ActivationFunctionType
    SCALE = 0.125
    G = 160

    nc = bacc.Bacc("TRN2", target_bir_lowering=False, debug=False)
    xt_d = [nc.dram_tensor(f"xt{b}", [768, 1025], F32R, kind="ExternalInput") for b in range(2)]
    wq_d = nc.dram_tensor("wqkv", [768, 2304], F32R, kind="ExternalInput")
    wp_d = nc.dram_tensor("wproj", [768, 768], F32R, kind="ExternalInput")
    bias_d = nc.dram_tensor("bias", [1, 768], F32, kind="ExternalInput")
    tpl_d = nc.dram_tensor("tpl", [128, 160], F32R, kind="ExternalInput")
    out_d = [nc.dram_tensor(f"out{b}", [1025, 768], F32, kind="ExternalOutput") for b in range(2)]

    with tile.TileContext(nc) as tc:
        with (
            tc.tile_pool(name="big", bufs=1) as big,
            tc.tile_pool(name="qwring", bufs=2) as qwring,
            tc.tile_pool(name="bigring", bufs=1) as bigring,
            tc.tile_pool(name="aug", bufs=2) as augring,
            tc.tile_pool(name="es", bufs=1) as es_pool,
            tc.tile_pool(name="nm", bufs=2) as nm_pool,
            tc.tile_pool(name="st", bufs=2) as st_pool,
            tc.tile_pool(name="row", bufs=1) as row_pool,
            tc.tile_pool(name="ps1", bufs=1, space="PSUM") as ps1,
            tc.tile_pool(name="pscls", bufs=2, space="PSUM") as pscls,
            tc.tile_pool(name="pso", bufs=2, space="PSUM") as pso,
            tc.tile_pool(name="psmm", bufs=2, space="PSUM") as psmm,
        ):
            # persistent
            vw = big.tile([128, 6, 768], F32R, tag="vw")  # W_qkv v-cols 1536:2304
            for k in range(6):
                nc.sync.dma_start(vw[:, k], wq_d.ap()[k * 128:(k + 1) * 128, 1536:2304])
            bias1 = big.tile([1, 768], F32, tag="bias1")
            nc.sync.dma_start(bias1[:], bias_d.ap())
            biasb = big.tile([128, 768], F32, tag="biasb")
            nc.sync.dma_start(biasb[:], bass.AP(bias1.tensor, bias1.offset, [[768, 1], [0, 128], [1, 768]]))
            ident = big.tile([128, 128], F32, tag="ident")
            make_identity(nc, ident[:])

            xt = big.tile([128, 6, 1025], F32R, tag="xt")
            qkt = big.tile([128, 12, 1024], F32R, tag="qkt")
            vstore = big.tile([128, 8, 6, G], F32R, tag="vstore")
            at = big.tile([128, 6, 1025], F32R, tag="at")
            x0col = big.tile([128, 6], F32R, tag="x0col")
            kct = [big.tile([128, 6], F32R, tag=f"kct{b}", name=f"kct{b}") for b in range(2)]
            q0blk = big.tile([128, 6, 12], F32R, tag="q0blk")
            q0cp = big.tile([128, 6], F32R, tag="q0cp")
            k0cp = big.tile([128, 6], F32R, tag="k0cp")
            w_cls = big.tile([12, 1025], F32, tag="wcls")
            wt_cls = big.tile([128, 9, 12], F32R, tag="wtcls")
            part = big.tile([12, 4], F32, tag="part")
            den_c = big.tile([12, 2], F32, tag="denc")
            clsnew = big.tile([128, 6], F32R, tag="clsnew")

            for t in range(8):
                for g in range(6):
                    nc.sync.dma_start(vstore[:, t, g], tpl_d.ap())

            VS_P = 8 * 6 * G

            def run_batch(b):
                # ---- stage A ----
                for k in range(6):
                    nc.sync.dma_start(xt[:, k], xt_d[b].ap()[k * 128:(k + 1) * 128, :])
                for k in range(6):
                    nc.vector.tensor_copy(x0col[:, k:k + 1], xt[:, k, 0:1])
                for m in range(12):
                    wm = qwring.tile([128, 6, 128], F32R, tag="qw")
                    for k in range(6):
                        nc.sync.dma_start(wm[:, k], wq_d.ap()[k * 128:(k + 1) * 128, m * 128:(m + 1) * 128])
                    for ci in range(2):
                        qk_ps = psmm.tile([128, 512], F32, tag="mm")
                        for k in range(6):
                            nc.tensor.matmul(qk_ps[:], wm[:, k], xt[:, k, 1 + ci * 512: 1 + (ci + 1) * 512],
                                             start=(k == 0), stop=(k == 5))
                        nc.vector.tensor_copy(qkt[:, m, ci * 512:(ci + 1) * 512], qk_ps[:])
                    if m < 6:
                        # q0 column for this feature tile (cls query)
                        q0_ps = pscls.tile([128, 512], F32, tag="cls")
                        for k in range(6):
                            nc.tensor.matmul(q0_ps[0:128, 0:1], wm[:, k].bitcast(F32), x0col[:, k:k + 1].bitcast(F32),
                                             start=(k == 0), stop=(k == 5))
                        nc.vector.tensor_copy(q0cp[:, m:m + 1], q0_ps[0:128, 0:1])
                for t in range(8):
                    for ci, (c0, cw, g0) in enumerate(((0, 512, 0), (512, 256, 4))):
                        v_ps = psmm.tile([128, 512], F32, tag="mm")
                        for k in range(6):
                            nc.tensor.matmul(v_ps[:, 0:cw], xt[:, k, 1 + t * 128: 1 + (t + 1) * 128],
                                             vw[:, k, c0:c0 + cw], start=(k == 0), stop=(k == 5))
                        dst = bass.AP(vstore.tensor, vstore.offset + t * 6 * G + g0 * G,
                                      [[VS_P, 128], [G, cw // 128], [96, 2], [1, 64]])
                        nc.vector.tensor_copy(dst, v_ps[:, 0:cw].rearrange("p (g h d) -> p g h d", h=2, d=64))

                # ---- k0 | v0 row of the original cls token ----
                k0v0 = row_pool.tile([1, 1536], F32, tag="rowbuf")
                for (dst0, cw, src_kind, s0) in ((0, 512, "stream", 768), (512, 256, "stream", 1280),
                                                 (768, 512, "vw", 0), (1280, 256, "vw", 512)):
                    if src_kind == "stream":
                        wr = bigring.tile([128, 6, 512], F32R, tag="bigr")
                        for k in range(6):
                            nc.sync.dma_start(wr[:, k, 0:cw], wq_d.ap()[k * 128:(k + 1) * 128, s0:s0 + cw])
                    r_ps = pscls.tile([128, 512], F32, tag="cls")
                    for k in range(6):
                        rhs = wr[:, k, 0:cw] if src_kind == "stream" else vw[:, k, s0:s0 + cw]
                        nc.tensor.matmul(r_ps[0:1, 0:cw], x0col[:, k:k + 1], rhs,
                                         start=(k == 0), stop=(k == 5))
                    nc.vector.tensor_copy(k0v0[0:1, dst0:dst0 + cw], r_ps[0:1, 0:cw])
                for k in range(6):
                    tp = pscls.tile([128, 512], F32, tag="cls")
                    nc.tensor.transpose(tp[0:128, 0:1], k0v0[0:1, k * 128:(k + 1) * 128], ident[0:1, 0:1])
                    nc.vector.tensor_copy(k0cp[:, k:k + 1], tp[0:128, 0:1])
                v0a = augring.tile([1, 6, G], F32R, tag="aug")
                for g in range(6):
                    nc.sync.dma_start(v0a[:, g], tpl_d.ap()[0:1, :])
                for (c0, ng, g0) in ((768, 4, 0), (1280, 2, 4)):
                    dst = bass.AP(v0a.tensor, v0a.offset + g0 * G, [[6 * G, 1], [G, ng], [96, 2], [1, 64]])
                    nc.vector.tensor_copy(dst, k0v0[0:1, c0:c0 + ng * 128].rearrange("p (g h d) -> p g h d", h=2, d=64))

                # ---- stage B: cls attention ----
                nc.sync.dma_start(q0blk.rearrange("p a c -> p (a c)")[:, 0:64], tpl_d.ap()[:, 0:64])
                nc.sync.dma_start(q0blk.rearrange("p a c -> p (a c)")[:, 64:72], tpl_d.ap()[:, 65:73])
                for k in range(6):
                    nc.vector.tensor_copy(q0blk[0:64, k, 2 * k:2 * k + 1], q0cp[0:64, k:k + 1])
                    nc.vector.tensor_copy(q0blk[64:128, k, 2 * k + 1:2 * k + 2], q0cp[64:128, k:k + 1])
                for ci in range(2):
                    cl_ps = pscls.tile([128, 512], F32, tag="cls")
                    for k in range(6):
                        nc.tensor.matmul(cl_ps[0:12, :], q0blk[:, k], qkt[:, 6 + k, ci * 512:(ci + 1) * 512],
                                         start=(k == 0), stop=(k == 5))
                    nc.scalar.activation(w_cls[:, 1 + ci * 512: 1 + (ci + 1) * 512], cl_ps[0:12, :],
                                         AF.Exp, scale=SCALE, accum_out=part[:, ci:ci + 1])
                cl0 = pscls.tile([128, 512], F32, tag="cls")
                for k in range(6):
                    nc.tensor.matmul(cl0[0:12, 0:1], q0blk[:, k].bitcast(F32), k0cp[:, k:k + 1].bitcast(F32),
                                     start=(k == 0), stop=(k == 5))
                nc.scalar.activation(w_cls[:, 0:1], cl0[0:12, 0:1], AF.Exp, scale=SCALE,
                                     accum_out=part[:, 2:3])
                nc.vector.tensor_add(den_c[:, 0:1], part[:, 0:1], part[:, 1:2])
                nc.vector.tensor_add(den_c[:, 0:1], den_c[:, 0:1], part[:, 2:3])
                nc.vector.reciprocal(den_c[:, 1:2], den_c[:, 0:1])
                nc.vector.tensor_scalar_mul(w_cls[:], w_cls[:], den_c[:, 1:2])
                t0 = pscls.tile([128, 512], F32, tag="cls")
                nc.tensor.transpose(t0[0:1, 0:12], w_cls[:, 0:1], ident[0:12, 0:12])
                nc.vector.tensor_copy(wt_cls[0:1, 0, :], t0[0:1, 0:12])
                for t in range(8):
                    tw = pscls.tile([128, 512], F32, tag="cls")
                    nc.tensor.transpose(tw[0:128, 0:12], w_cls[:, 1 + t * 128: 1 + (t + 1) * 128],
                                        ident[0:12, 0:12])
                    nc.vector.tensor_copy(wt_cls[:, 1 + t, :], tw[0:128, 0:12])
                for fi in range(6):
                    co_a = pscls.tile([128, 512], F32, tag="cls")
                    co_b = pscls.tile([128, 512], F32, tag="cls")
                    lA0 = bass.AP(v0a.tensor, v0a.offset + fi * G, [[6 * G, 1], [1, 64]])
                    nc.tensor.matmul(co_a[0:64, 0:12], lA0, wt_cls[0:1, 0, :], start=True, stop=False)
                    lB0 = bass.AP(v0a.tensor, v0a.offset + fi * G + 32, [[6 * G, 1], [1, 128]])
                    nc.tensor.matmul(co_b[0:128, 0:12], lB0, wt_cls[0:1, 0, :], start=True, stop=False)
                    for t in range(8):
                        lA = bass.AP(vstore.tensor, vstore.offset + t * 6 * G + fi * G,
                                     [[VS_P, 128], [1, 64]])
                        nc.tensor.matmul(co_a[0:64, 0:12], lA, wt_cls[:, 1 + t, :],
                                         start=False, stop=(t == 7))
                        lB = bass.AP(vstore.tensor, vstore.offset + t * 6 * G + fi * G + 32,
                                     [[VS_P, 128], [1, 128]])
                        nc.tensor.matmul(co_b[0:128, 0:12], lB, wt_cls[:, 1 + t, :],
                                         start=False, stop=(t == 7))
                    nc.vector.tensor_add(at[0:64, fi, 0:1], co_a[0:64, 2 * fi:2 * fi + 1], x0col[0:64, fi:fi + 1])
                    nc.vector.tensor_add(at[64:128, fi, 0:1], co_b[64:128, 2 * fi + 1:2 * fi + 2], x0col[64:128, fi:fi + 1])

                # ---- stage C: updated cls k, v ----
                for k in range(6):
                    nc.vector.tensor_copy(clsnew[:, k:k + 1], at[:, k, 0:1])
                kvc = row_pool.tile([1, 1536], F32, tag="rowbuf")
                for (dst0, s0) in ((0, 768), (512, 1280), (1024, 1792)):
                    wr = bigring.tile([128, 6, 512], F32R, tag="bigr")
                    for k in range(6):
                        nc.sync.dma_start(wr[:, k], wq_d.ap()[k * 128:(k + 1) * 128, s0:s0 + 512])
                    kv_ps = pscls.tile([128, 512], F32, tag="cls")
                    for k in range(6):
                        nc.tensor.matmul(kv_ps[0:1, :], clsnew[:, k:k + 1], wr[:, k],
                                         start=(k == 0), stop=(k == 5))
                    nc.vector.tensor_copy(kvc[0:1, dst0:dst0 + 512], kv_ps[0:1, :])
                for k in range(6):
                    tk = pscls.tile([128, 512], F32, tag="cls")
                    nc.tensor.transpose(tk[0:128, 0:1], kvc[0:1, k * 128:(k + 1) * 128], ident[0:1, 0:1])
                    nc.vector.tensor_copy(kct[b][:, k:k + 1], tk[0:128, 0:1])
                vca = augring.tile([1, 6, G], F32R, tag="aug")
                for g in range(6):
                    nc.sync.dma_start(vca[:, g], tpl_d.ap()[0:1, :])
                for (c0, ng, g0) in ((768, 4, 0), (1280, 2, 4)):
                    dst = bass.AP(vca.tensor, vca.offset + g0 * G, [[6 * G, 1], [G, ng], [96, 2], [1, 64]])
                    nc.vector.tensor_copy(dst, kvc[0:1, c0:c0 + ng * 128].rearrange("p (g h d) -> p g h d", h=2, d=64))

                # ---- stage D: branch attention ----
                for br in range(4):
                    for fi in range(6):
                        qsl = slice(br * 256, (br + 1) * 256)
                        ps_sa = ps1.tile([128, 512], F32, tag="sa")
                        ps_sb = ps1.tile([128, 512], F32, tag="sb")
                        for half in range(2):
                            ksl = slice(br * 256 + half * 128, br * 256 + (half + 1) * 128)
                            nc.tensor.matmul(ps_sa[:, half * 256:(half + 1) * 256],
                                             qkt[0:64, 6 + fi, ksl], qkt[0:64, fi, qsl],
                                             start=True, stop=True)
                            nc.tensor.matmul(ps_sb[:, half * 256:(half + 1) * 256],
                                             qkt[64:128, 6 + fi, ksl], qkt[64:128, fi, qsl],
                                             start=True, stop=True)
                        ps_ca = pscls.tile([128, 512], F32, tag="cls")
                        ps_cb = pscls.tile([128, 512], F32, tag="cls")
                        nc.tensor.matmul(ps_ca[0:1, 0:256], kct[b][0:64, fi:fi + 1], qkt[0:64, fi, qsl],
                                         start=True, stop=True)
                        nc.tensor.matmul(ps_cb[0:1, 0:256], kct[b][64:128, fi:fi + 1], qkt[64:128, fi, qsl],
                                         start=True, stop=True)
                        esa = es_pool.tile([128, 512], F32R, tag="esa")
                        esb = es_pool.tile([128, 512], F32R, tag="esb")
                        esc = es_pool.tile([1, 512], F32R, tag="esc")
                        nc.scalar.activation(esa[:], ps_sa[:], AF.Exp, scale=SCALE)
                        nc.scalar.activation(esb[:], ps_sb[:], AF.Exp, scale=SCALE)
                        nc.scalar.activation(esc[0:1, 0:256], ps_ca[0:1, 0:256], AF.Exp, scale=SCALE)
                        nc.scalar.activation(esc[0:1, 256:512], ps_cb[0:1, 0:256], AF.Exp, scale=SCALE)
                        ps_oa = pso.tile([128, 256], F32, tag="o")
                        ps_ob = psmm.tile([128, 256], F32, tag="mm")
                        lhsA0 = bass.AP(vca.tensor, vca.offset + fi * G, [[6 * G, 1], [1, 65]])
                        nc.tensor.matmul(ps_oa[0:65, :], lhsA0, esc[0:1, 0:256], start=True, stop=False)
                        lhsB0 = bass.AP(vca.tensor, vca.offset + fi * G + 32, [[6 * G, 1], [1, 128]])
                        nc.tensor.matmul(ps_ob[0:128, :], lhsB0, esc[0:1, 256:512], start=True, stop=False)
                        for half in range(2):
                            tt = 2 * br + half
                            lhsA = bass.AP(vstore.tensor, vstore.offset + tt * 6 * G + fi * G,
                                           [[VS_P, 128], [1, 65]])
                            nc.tensor.matmul(ps_oa[0:65, :], lhsA, esa[:, half * 256:(half + 1) * 256],
                                             start=False, stop=(half == 1))
                            lhsB = bass.AP(vstore.tensor, vstore.offset + tt * 6 * G + fi * G + 32,
                                           [[VS_P, 128], [1, 128]])
                            nc.tensor.matmul(ps_ob[0:128, :], lhsB, esb[:, half * 256:(half + 1) * 256],
                                             start=False, stop=(half == 1))
                        # stage denominator rows to SBUF, DMA-broadcast, then wide reciprocal
                        ra = nm_pool.tile([128, 256], F32, tag="ra")
                        nc.vector.tensor_copy(ra[64:65, :], ps_oa[64:65, :])
                        nc.vector.tensor_copy(ra[32:33, :], ps_ob[32:33, :])
                        rb = nm_pool.tile([128, 256], F32, tag="rb")
                        nc.sync.dma_start(rb[0:64, :], bass.AP(ra.tensor, ra.offset + 64 * 256,
                                                               [[256, 1], [0, 64], [1, 256]]))
                        nc.sync.dma_start(rb[64:128, :], bass.AP(ra.tensor, ra.offset + 32 * 256,
                                                                 [[256, 1], [0, 64], [1, 256]]))
                        nc.vector.reciprocal(rb[:, :], rb[:, :])
                        csl = slice(1 + br * 256, 1 + (br + 1) * 256)
                        nc.vector.tensor_mul(at[0:64, fi, csl], ps_oa[0:64, :], rb[0:64, :])
                        nc.vector.tensor_mul(at[64:128, fi, csl], ps_ob[64:128, :], rb[64:128, :])

                # ---- stage E: projection ----
                for (c0, cw) in ((0, 512), (512, 256)):
                    wpc = bigring.tile([128, 6, 512], F32R, tag="bigr")
                    for k in range(6):
                        nc.sync.dma_start(wpc[:, k, 0:cw], wp_d.ap()[k * 128:(k + 1) * 128, c0:c0 + cw])
                    for mt in range(9):
                        m0, mw = (mt * 128, 128) if mt < 8 else (1024, 1)
                        pr = psmm.tile([128, 512], F32, tag="mm")
                        for k in range(6):
                            nc.tensor.matmul(pr[0:mw, 0:cw], at[:, k, m0:m0 + mw], wpc[:, k, 0:cw],
                                             start=(k == 0), stop=(k == 5))
                        stg = st_pool.tile([128, 512], F32, tag="stg")
                        nc.vector.tensor_add(stg[0:mw, 0:cw], pr[0:mw, 0:cw], biasb[0:mw, c0:c0 + cw])
                        nc.sync.dma_start(out_d[b].ap()[m0:m0 + mw, c0:c0 + cw], stg[0:mw, 0:cw])

            run_batch(0)
            run_batch(1)

    nc.compile()
    _NC_CACHE["nc"] = nc
    return nc


def kernel(x, W_qkv, W_proj, b_proj):
    _ensure_ntff_hook()
    from concourse import bass_utils
    x = np.asarray(x, dtype=np.float32)
    W_qkv = np.asarray(W_qkv, dtype=np.float32)
    W_proj = np.asarray(W_proj, dtype=np.float32)
    b_proj = np.asarray(b_proj, dtype=np.float32)

    nc = build_program()
    xt = np.ascontiguousarray(np.transpose(x, (0, 2, 1)))
    tpl = np.zeros((128, 160), np.float32)
    tpl[:, 64] = 1.0
    bias = np.ascontiguousarray(b_proj.reshape(1, 768))
    in_maps = [{"xt0": xt[2 * c], "xt1": xt[2 * c + 1],
                "wqkv": W_qkv, "wproj": W_proj, "bias": bias, "tpl": tpl}
               for c in range(8)]
    res = bass_utils.run_bass_kernel_spmd(nc, in_maps, list(range(8)))
    out = np.empty((16, 1025, 768), np.float32)
    for c in range(8):
        out[2 * c] = res.results[c]["out0"]
        out[2 * c + 1] = res.results[c]["out1"]
    return out



# revision 14
# speedup vs baseline: 1.3040x; 1.3040x over previous
"""Trainium2 Bass kernel for nn_Attention_Sep (sparse attention, B=16 N=1025 C=768 H=12 nb=4).

v2: all-bf16 data path (host-cast inputs), persistent weights in SBUF, dense PE
scheduling with batch interleaving via disjoint per-batch buffers.

Data-parallel over batch (2 per core, 8 cores). Per core, transposed
[feature, token] layout:
  A) Q^T/K^T feature-tiles = W_qkv.T @ x^T (bf16); V natural [token, feature]
     stored in 132-wide head-pair groups [A(64)|onesA|B(64)|onesB] so the PV
     matmul emits softmax denominators on PSUM row 64 for both heads.
  B) cls attention over all 1025 tokens: block-diagonal q0 lhsT -> [12, tok]
     logits, row softmax on ACT (exp+accum), PE-transposed weights, V
     contraction, residual -> clsat.
  C) recompute k,v of the updated cls token (row matmuls + tiny transposes).
  D) cls-key logits hoisted per head-pair via block lhsT [kcA|0..|kcB] (out
     rows 0/64); per (branch, pair): S^T QK, exp (ACT, scale 1/8), PV with
     denominators at row 64; reciprocal_approx_fast + one broadcast DMA + DVE
     muls -> per-branch A^T tile.
  E) projection per branch (as soon as the branch finishes): bias folded into
     the matmul via a ones-row, PSUM->SBUF copy, DMA out.
"""
import sys, types
import numpy as np


def _ensure_ntff_hook():
    try:
        import antenv
        if "antenv.axon_hooks" in sys.modules:
            return
        from trn_agent_boot.trn_boot import _ntff_profile_via_ctypes
        mod = types.ModuleType("antenv.axon_hooks")
        mod._hook = None
        mod.set_axon_ntff_profile_hook = lambda h: setattr(mod, "_hook", h)
        mod.get_axon_ntff_profile_hook = lambda: mod._hook
        sys.modules["antenv.axon_hooks"] = mod
        antenv.axon_hooks = mod
        mod.set_axon_ntff_profile_hook(_ntff_profile_via_ctypes('/opt/axon/libaxon_pjrt.so'))
    except Exception:
        pass


_NC_CACHE = {}

G = 132          # head-pair group width in vstore: [A(0:64)|onesA(64)|B(65:129)|onesB(129)|pad]
ASL = (0, 65)    # A-slice of a group (A feats + onesA)
BSL = (65, 130)  # B-slice (B feats + onesB), feats->out rows 0:64, den->row 64


def build_program():
    if "nc" in _NC_CACHE:
        return _NC_CACHE["nc"]
    import concourse.bass as bass
    import concourse.mybir as mybir
    import concourse.tile as tile
    from concourse import bacc
    from concourse.masks import make_identity

    F32, BF16 = mybir.dt.float32, mybir.dt.bfloat16
    AF = mybir.ActivationFunctionType
    SCALE = 0.125

    nc = bacc.Bacc("TRN2", target_bir_lowering=False, debug=False)
    xt_d = [nc.dram_tensor(f"xt{b}", [768, 1025], BF16, kind="ExternalInput") for b in range(2)]
    wq_d = nc.dram_tensor("wqkv", [768, 2304], BF16, kind="ExternalInput")
    wp_d = nc.dram_tensor("wproj", [768, 768], BF16, kind="ExternalInput")
    bias_d = nc.dram_tensor("bias", [1, 768], BF16, kind="ExternalInput")
    out_d = [nc.dram_tensor(f"out{b}", [1025, 768], F32, kind="ExternalOutput") for b in range(2)]
    DEBUG = bool(__import__("os").environ.get("KDBG"))
    if DEBUG:
        dbg_qkt = [nc.dram_tensor(f"dqkt{b}", [128, 12 * 1024], mybir.dt.bfloat16, kind="ExternalOutput") for b in range(2)]
        dbg_cls = [nc.dram_tensor(f"dcls{b}", [128, 6], mybir.dt.bfloat16, kind="ExternalOutput") for b in range(2)]
        dbg_esc = [nc.dram_tensor(f"desc{b}", [2, 6 * 1024], mybir.dt.bfloat16, kind="ExternalOutput") for b in range(2)]
        dbg_kvc = [nc.dram_tensor(f"dkvc{b}", [1, 1536], mybir.dt.bfloat16, kind="ExternalOutput") for b in range(2)]
        dbg_atb = [nc.dram_tensor(f"datb{b}", [128, 6 * 256], mybir.dt.bfloat16, kind="ExternalOutput") for b in range(2)]
        dbg_vst = [nc.dram_tensor(f"dvst{b}", [128, 6 * 130], mybir.dt.bfloat16, kind="ExternalOutput") for b in range(2)]

    with tile.TileContext(nc) as tc:
        with (
            tc.tile_pool(name="big", bufs=1) as big,
            tc.tile_pool(name="pb", bufs=1) as pb,
            tc.tile_pool(name="xe", bufs=2) as xe,      # xt / esc alias ring
            tc.tile_pool(name="es", bufs=4) as es_pool,
            tc.tile_pool(name="rb", bufs=2) as rb_pool,
            tc.tile_pool(name="bb", bufs=2) as bb_pool,
            tc.tile_pool(name="st", bufs=2) as st_pool,
            tc.tile_pool(name="ab", bufs=2) as ab_pool,
            tc.tile_pool(name="row", bufs=2) as row_pool,
            tc.tile_pool(name="pss", bufs=2, space="PSUM") as pss,   # [128,1024] S^T  -> 4 banks
            tc.tile_pool(name="pso", bufs=2, space="PSUM") as pso,   # [128,512]  PV   -> 2 banks
            tc.tile_pool(name="psw", bufs=2, space="PSUM") as psw,   # [128,512]  misc -> 2 banks
        ):
            # ---------- persistent weights ----------
            wq_r = wq_d.ap().rearrange("(k p) c -> p k c", p=128)
            wqk = big.tile([128, 6, 1536], BF16, tag="wqk")
            for half in range(2):
                nc.sync.dma_start(
                    wqk[:, :, half * 768:(half + 1) * 768],
                    wq_r[:, :, half * 768:(half + 1) * 768])
            wv = big.tile([128, 6, 768], BF16, tag="wv")
            nc.sync.dma_start(wv[:], wq_r[:, :, 1536:2304])
            wp = big.tile([128, 6, 768], BF16, tag="wp")
            nc.sync.dma_start(wp[:], wp_d.ap().rearrange("(k p) c -> p k c", p=128))
            biasr = big.tile([1, 768], BF16, tag="biasr")
            nc.sync.dma_start(biasr[:], bias_d.ap())
            ident = big.tile([128, 128], BF16, tag="ident")
            make_identity(nc, ident[:])
            onesrow = big.tile([1, 128], BF16, tag="onesrow")
            nc.vector.memset(onesrow[:], 1.0)

            # ---------- per-batch tiles ----------
            qkt = [pb.tile([128, 12, 1024], BF16, tag=f"qkt{b}", name=f"qkt{b}") for b in range(2)]
            vst = [pb.tile([128, 8, 6, G], BF16, tag=f"vst{b}", name=f"vst{b}") for b in range(2)]
            x0col = [pb.tile([128, 6], BF16, tag=f"x0c{b}", name=f"x0c{b}") for b in range(2)]
            q0cp = [pb.tile([128, 6], BF16, tag=f"q0c{b}", name=f"q0c{b}") for b in range(2)]
            k0cp = [pb.tile([128, 6], BF16, tag=f"k0c{b}", name=f"k0c{b}") for b in range(2)]
            kct = [pb.tile([128, 6], BF16, tag=f"kct{b}", name=f"kct{b}") for b in range(2)]
            clsat = [pb.tile([128, 6], BF16, tag=f"ca{b}", name=f"ca{b}") for b in range(2)]
            q0blk = [pb.tile([128, 6, 12], BF16, tag=f"q0b{b}", name=f"q0b{b}") for b in range(2)]
            kblk = [pb.tile([128, 6, 65], BF16, tag=f"kb{b}", name=f"kb{b}") for b in range(2)]
            wclsb = [pb.tile([12, 1025], BF16, tag=f"wc{b}", name=f"wc{b}") for b in range(2)]
            wtcls = [pb.tile([128, 9, 12], BF16, tag=f"wt{b}", name=f"wt{b}") for b in range(2)]
            part = [pb.tile([12, 4], F32, tag=f"pt{b}", name=f"pt{b}") for b in range(2)]
            v0a = [pb.tile([65, 6, G], BF16, tag=f"v0a{b}", name=f"v0a{b}") for b in range(2)]
            vca = [pb.tile([65, 6, G], BF16, tag=f"vca{b}", name=f"vca{b}") for b in range(2)]

            # ones columns of vstore groups (once per batch buffer)
            for b in range(2):
                nc.vector.memset(
                    bass.AP(vst[b].tensor, vst[b].offset + 64,
                            [[8 * 6 * G, 128], [6 * G, 8], [G, 6], [65, 2]]), 1.0)
                for t in (v0a[b], vca[b]):
                    nc.vector.memset(
                        bass.AP(t.tensor, t.offset + 64, [[6 * G, 1], [G, 6], [1, 1]]), 1.0)
                    nc.vector.memset(
                        bass.AP(t.tensor, t.offset + 64 * 6 * G + 129, [[6 * G, 1], [G, 6], [1, 1]]), 1.0)
                nc.vector.memset(q0blk[b][:], 0.0)
                nc.vector.memset(kblk[b][:], 0.0)

            VS_P = 8 * 6 * G

            def run_batch(b):
                # ================= stage A =================
                xt = xe.tile([128, 6, 1025], BF16, tag="xtesc")
                nc.sync.dma_start(xt[:], xt_d[b].ap().rearrange("(k p) n -> p k n", p=128))
                nc.vector.tensor_copy(
                    x0col[b][:],
                    bass.AP(xt.tensor, xt.offset, [[6 * 1025, 128], [1025, 6]]))
                # Q^T, K^T feature tiles
                for m in range(12):
                    for ci in range(2):
                        qk_ps = psw.tile([128, 512], F32, tag="w2")
                        for k in range(6):
                            nc.tensor.matmul(qk_ps[:], wqk[:, k, m * 128:(m + 1) * 128],
                                             xt[:, k, 1 + ci * 512: 1 + (ci + 1) * 512],
                                             start=(k == 0), stop=(k == 5))
                        if ci == 0:
                            nc.vector.tensor_copy(qkt[b][:, m, 0:512], qk_ps[:])
                        else:
                            nc.scalar.copy(qkt[b][:, m, 512:1024], qk_ps[:])
                # V natural into grouped layout
                for t in range(8):
                    for ci, (c0, cw, g0) in enumerate(((0, 512, 0), (512, 256, 4))):
                        v_ps = psw.tile([128, 512], F32, tag="w2")
                        for k in range(6):
                            nc.tensor.matmul(v_ps[:, 0:cw], xt[:, k, 1 + t * 128: 1 + (t + 1) * 128],
                                             wv[:, k, c0:c0 + cw], start=(k == 0), stop=(k == 5))
                        dst = bass.AP(vst[b].tensor, vst[b].offset + t * 6 * G + g0 * G,
                                      [[VS_P, 128], [G, cw // 128], [65, 2], [1, 64]])
                        nc.vector.tensor_copy(dst, v_ps[:, 0:cw].rearrange("p (g h d) -> p g h d", h=2, d=64))
                # q0|k0|v0 row of the original cls token
                qkv0 = row_pool.tile([1, 2304], BF16, tag="rowq")
                for (c0, cw) in ((0, 512), (512, 512), (1024, 512), (1536, 512), (2048, 256)):
                    r_ps = psw.tile([128, 512], F32, tag="w2")
                    for k in range(6):
                        rhs = wqk[:, k, c0:c0 + cw] if c0 < 1536 else wv[:, k, c0 - 1536:c0 - 1536 + cw]
                        nc.tensor.matmul(r_ps[0:1, 0:cw], x0col[b][:, k:k + 1], rhs,
                                         start=(k == 0), stop=(k == 5))
                    nc.vector.tensor_copy(qkv0[0:1, c0:c0 + cw], r_ps[0:1, 0:cw])
                for k in range(6):
                    tq = psw.tile([128, 512], BF16, tag="w2")
                    nc.tensor.transpose(tq[0:128, 0:1], qkv0[0:1, k * 128:(k + 1) * 128], ident[0:1, 0:1])
                    nc.vector.tensor_copy(q0cp[b][:, k:k + 1], tq[0:128, 0:1])
                    tk = psw.tile([128, 512], BF16, tag="w2")
                    nc.tensor.transpose(tk[0:128, 0:1], qkv0[0:1, 768 + k * 128: 768 + (k + 1) * 128],
                                        ident[0:1, 0:1])
                    nc.vector.tensor_copy(k0cp[b][:, k:k + 1], tk[0:128, 0:1])
                # v0 -> aug groups (A at row 0, B at row 64)
                nc.vector.tensor_copy(
                    bass.AP(v0a[b].tensor, v0a[b].offset, [[6 * G, 1], [G, 6], [1, 64]]),
                    bass.AP(qkv0.tensor, qkv0.offset + 1536, [[2304, 1], [128, 6], [1, 64]]))
                nc.vector.tensor_copy(
                    bass.AP(v0a[b].tensor, v0a[b].offset + 64 * 6 * G + 65, [[6 * G, 1], [G, 6], [1, 64]]),
                    bass.AP(qkv0.tensor, qkv0.offset + 1600, [[2304, 1], [128, 6], [1, 64]]))

                # ================= stage B: cls attention =================
                nc.vector.tensor_copy(
                    bass.AP(q0blk[b].tensor, q0blk[b].offset, [[6 * 12, 64], [14, 6]]),
                    bass.AP(q0cp[b].tensor, q0cp[b].offset, [[6, 64], [1, 6]]))
                nc.vector.tensor_copy(
                    bass.AP(q0blk[b].tensor, q0blk[b].offset + 64 * 6 * 12 + 1, [[6 * 12, 64], [14, 6]]),
                    bass.AP(q0cp[b].tensor, q0cp[b].offset + 64 * 6, [[6, 64], [1, 6]]))
                for ci in range(2):
                    cl_ps = psw.tile([128, 512], F32, tag="w2")
                    for k in range(6):
                        nc.tensor.matmul(cl_ps[0:12, :], q0blk[b][:, k], qkt[b][:, 6 + k, ci * 512:(ci + 1) * 512],
                                         start=(k == 0), stop=(k == 5))
                    nc.scalar.activation(wclsb[b][:, 1 + ci * 512: 1 + (ci + 1) * 512], cl_ps[0:12, :],
                                         AF.Exp, scale=SCALE, accum_out=part[b][:, ci:ci + 1])
                cl0 = psw.tile([128, 512], F32, tag="w2")
                for k in range(6):
                    nc.tensor.matmul(cl0[0:12, 0:1], q0blk[b][:, k], k0cp[b][:, k:k + 1],
                                     start=(k == 0), stop=(k == 5))
                nc.scalar.activation(wclsb[b][:, 0:1], cl0[0:12, 0:1], AF.Exp, scale=SCALE,
                                     accum_out=part[b][:, 2:3])
                nc.vector.tensor_add(part[b][:, 3:4], part[b][:, 0:1], part[b][:, 1:2])
                nc.vector.tensor_add(part[b][:, 3:4], part[b][:, 3:4], part[b][:, 2:3])
                nc.vector.reciprocal(part[b][:, 3:4], part[b][:, 3:4])
                nc.vector.tensor_scalar_mul(wclsb[b][:], wclsb[b][:], part[b][:, 3:4])
                t0 = psw.tile([128, 512], BF16, tag="w2")
                nc.tensor.transpose(t0[0:1, 0:12], wclsb[b][:, 0:1], ident[0:12, 0:12])
                nc.vector.tensor_copy(wtcls[b][0:1, 0, :], t0[0:1, 0:12])
                nc.vector.tensor_copy(wtcls[b][64:65, 0, :], t0[0:1, 0:12])
                for t in range(8):
                    tw = psw.tile([128, 512], BF16, tag="w2")
                    nc.tensor.transpose(tw[0:128, 0:12], wclsb[b][:, 1 + t * 128: 1 + (t + 1) * 128],
                                        ident[0:12, 0:12])
                    nc.vector.tensor_copy(wtcls[b][:, 1 + t, :], tw[0:128, 0:12])
                for fi in range(6):
                    co_a = pso.tile([128, 512], F32, tag="o")
                    co_b = pso.tile([128, 512], F32, tag="o")
                    lA0 = bass.AP(v0a[b].tensor, v0a[b].offset + fi * G, [[6 * G, 1], [1, 65]])
                    nc.tensor.matmul(co_a[0:65, 0:12], lA0, wtcls[b][0:1, 0, :], start=True, stop=False)
                    lB0 = bass.AP(v0a[b].tensor, v0a[b].offset + 64 * 6 * G + fi * G + 65, [[6 * G, 1], [1, 65]])
                    nc.tensor.matmul(co_b[0:65, 0:12], lB0, wtcls[b][64:65, 0, :], start=True, stop=False)
                    for t in range(8):
                        lA = bass.AP(vst[b].tensor, vst[b].offset + t * 6 * G + fi * G, [[VS_P, 128], [1, 65]])
                        nc.tensor.matmul(co_a[0:65, 0:12], lA, wtcls[b][:, 1 + t, :],
                                         start=False, stop=(t == 7))
                        lB = bass.AP(vst[b].tensor, vst[b].offset + t * 6 * G + fi * G + 65, [[VS_P, 128], [1, 65]])
                        nc.tensor.matmul(co_b[0:65, 0:12], lB, wtcls[b][:, 1 + t, :],
                                         start=False, stop=(t == 7))
                    nc.vector.tensor_add(clsat[b][0:64, fi:fi + 1], co_a[0:64, 2 * fi:2 * fi + 1],
                                         x0col[b][0:64, fi:fi + 1])
                    nc.vector.tensor_add(clsat[b][64:128, fi:fi + 1], co_b[0:64, 2 * fi + 1:2 * fi + 2],
                                         x0col[b][64:128, fi:fi + 1])

                # ---- cls output row (projection of clsat) ----
                stgc = st_pool.tile([128, 768], F32, tag="stg")
                for (c0, cw) in ((0, 512), (512, 256)):
                    pr = psw.tile([128, 512], F32, tag="w2")
                    nc.tensor.matmul(pr[0:1, 0:cw], onesrow[0:1, 0:1], biasr[0:1, c0:c0 + cw],
                                     start=True, stop=False)
                    for k in range(6):
                        nc.tensor.matmul(pr[0:1, 0:cw], clsat[b][:, k:k + 1], wp[:, k, c0:c0 + cw],
                                         start=False, stop=(k == 5))
                    nc.vector.tensor_copy(stgc[0:1, c0:c0 + cw], pr[0:1, 0:cw])
                nc.sync.dma_start(out_d[b].ap()[0:1, :], stgc[0:1, :])

                # ================= stage C: updated cls k, v =================
                kvc = row_pool.tile([1, 1536], BF16, tag="rowq")
                for (c0, cw) in ((0, 512), (512, 256), (768, 512), (1280, 256)):
                    kv_ps = psw.tile([128, 512], F32, tag="w2")
                    for k in range(6):
                        rhs = wqk[:, k, 768 + c0:768 + c0 + cw] if c0 < 768 else wv[:, k, c0 - 768:c0 - 768 + cw]
                        nc.tensor.matmul(kv_ps[0:1, 0:cw], clsat[b][:, k:k + 1], rhs,
                                         start=(k == 0), stop=(k == 5))
                    nc.vector.tensor_copy(kvc[0:1, c0:c0 + cw], kv_ps[0:1, 0:cw])
                for k in range(6):
                    tk = psw.tile([128, 512], BF16, tag="w2")
                    nc.tensor.transpose(tk[0:128, 0:1], kvc[0:1, k * 128:(k + 1) * 128], ident[0:1, 0:1])
                    nc.vector.tensor_copy(kct[b][:, k:k + 1], tk[0:128, 0:1])
                # kblk: [kcA | 0..0 | kcB] block columns (col 0 rows 0:64, col 64 rows 64:128)
                nc.vector.tensor_copy(
                    bass.AP(kblk[b].tensor, kblk[b].offset, [[6 * 65, 64], [65, 6]]),
                    bass.AP(kct[b].tensor, kct[b].offset, [[6, 64], [1, 6]]))
                nc.vector.tensor_copy(
                    bass.AP(kblk[b].tensor, kblk[b].offset + 64 * 6 * 65 + 64, [[6 * 65, 64], [65, 6]]),
                    bass.AP(kct[b].tensor, kct[b].offset + 64 * 6, [[6, 64], [1, 6]]))
                # vca aug groups
                nc.vector.tensor_copy(
                    bass.AP(vca[b].tensor, vca[b].offset, [[6 * G, 1], [G, 6], [1, 64]]),
                    bass.AP(kvc.tensor, kvc.offset + 768, [[1536, 1], [128, 6], [1, 64]]))
                nc.vector.tensor_copy(
                    bass.AP(vca[b].tensor, vca[b].offset + 64 * 6 * G + 65, [[6 * G, 1], [G, 6], [1, 64]]),
                    bass.AP(kvc.tensor, kvc.offset + 832, [[1536, 1], [128, 6], [1, 64]]))

                # ================= stage D: branch attention =================
                # hoisted cls-key logits: esc rows 0 (head A) / 64 (head B)
                esc = xe.tile([128, 6, 1024], BF16, tag="xtesc")
                for fi in range(6):
                    cpa = pss.tile([128, 1024], F32, tag="s")
                    cpb = pss.tile([128, 1024], F32, tag="s")
                    for ci in range(2):
                        nc.tensor.matmul(cpa[0:1, ci * 512:(ci + 1) * 512], kblk[b][:, fi, 0:1],
                                         qkt[b][:, fi, ci * 512:(ci + 1) * 512], start=True, stop=True)
                        nc.tensor.matmul(cpb[64:65, ci * 512:(ci + 1) * 512], kblk[b][:, fi, 64:65],
                                         qkt[b][:, fi, ci * 512:(ci + 1) * 512], start=True, stop=True)
                    nc.scalar.activation(esc[0:1, fi, :], cpa[0:1, :], AF.Exp, scale=SCALE)
                    nc.scalar.activation(esc[64:65, fi, :], cpb[64:65, :], AF.Exp, scale=SCALE)
                if DEBUG:
                    nc.sync.dma_start(dbg_qkt[b].ap(), qkt[b].rearrange("p a c -> p (a c)"))
                    nc.sync.dma_start(dbg_cls[b].ap(), clsat[b][:])
                    nc.sync.dma_start(dbg_esc[b].ap()[0:1, :], esc[0:1].rearrange("p a c -> p (a c)"))
                    nc.sync.dma_start(dbg_esc[b].ap()[1:2, :], esc[64:65].rearrange("p a c -> p (a c)"))
                    nc.sync.dma_start(dbg_kvc[b].ap(), kvc[:])
                    nc.sync.dma_start(dbg_vst[b].ap(), bass.AP(vst[b].tensor, vst[b].offset, [[VS_P, 128], [G, 6], [1, 130]]))
                for br in range(4):
                    atb = ab_pool.tile([128, 6, 256], BF16, tag="atb")
                    for fi in range(6):
                        qsl = slice(br * 256, (br + 1) * 256)
                        ps_s = pss.tile([128, 1024], F32, tag="s")
                        for half in range(2):
                            ksl = slice(br * 256 + half * 128, br * 256 + (half + 1) * 128)
                            nc.tensor.matmul(ps_s[:, half * 256:(half + 1) * 256],
                                             qkt[b][0:64, 6 + fi, ksl], qkt[b][0:64, fi, qsl],
                                             start=True, stop=True)
                            nc.tensor.matmul(ps_s[:, 512 + half * 256: 512 + (half + 1) * 256],
                                             qkt[b][64:128, 6 + fi, ksl], qkt[b][64:128, fi, qsl],
                                             start=True, stop=True)
                        esa = es_pool.tile([128, 512], BF16, tag="es")
                        esb = es_pool.tile([128, 512], BF16, tag="es")
                        nc.scalar.activation(esa[:], ps_s[:, 0:512], AF.Exp, scale=SCALE)
                        nc.scalar.activation(esb[:], ps_s[:, 512:1024], AF.Exp, scale=SCALE)
                        ps_o = pso.tile([128, 512], F32, tag="o")
                        lA0 = bass.AP(vca[b].tensor, vca[b].offset + fi * G, [[6 * G, 1], [1, 65]])
                        nc.tensor.matmul(ps_o[0:65, 0:256], lA0, esc[0:1, fi, qsl], start=True, stop=False)
                        for half in range(2):
                            tt = 2 * br + half
                            lA = bass.AP(vst[b].tensor, vst[b].offset + tt * 6 * G + fi * G, [[VS_P, 128], [1, 65]])
                            nc.tensor.matmul(ps_o[0:65, 0:256], lA, esa[:, half * 256:(half + 1) * 256],
                                             start=False, stop=(half == 1))
                        lB0 = bass.AP(vca[b].tensor, vca[b].offset + 64 * 6 * G + fi * G + 65, [[6 * G, 1], [1, 65]])
                        nc.tensor.matmul(ps_o[0:65, 256:512], lB0, esc[64:65, fi, qsl], start=True, stop=False)
                        for half in range(2):
                            tt = 2 * br + half
                            lB = bass.AP(vst[b].tensor, vst[b].offset + tt * 6 * G + fi * G + 65, [[VS_P, 128], [1, 65]])
                            nc.tensor.matmul(ps_o[0:65, 256:512], lB, esb[:, half * 256:(half + 1) * 256],
                                             start=False, stop=(half == 1))
                        rb = rb_pool.tile([1, 512], F32, tag="rb")
                        nc.vector.tensor_copy(rb[0:1, :], ps_o[64:65, :])
                        nc.vector.reciprocal_approx_fast(rb[0:1, :], rb[0:1, :])
                        rbb = bb_pool.tile([64, 512], F32, tag="rbb")
                        nc.sync.dma_start(rbb[0:64, :], bass.AP(rb.tensor, rb.offset,
                                                                [[512, 1], [0, 64], [1, 512]]))
                        nc.vector.tensor_mul(atb[0:64, fi, :], ps_o[0:64, 0:256], rbb[0:64, 0:256])
                        nc.vector.tensor_mul(atb[64:128, fi, :], ps_o[0:64, 256:512], rbb[0:64, 256:512])
                    if DEBUG and br == 0:
                        nc.sync.dma_start(dbg_atb[b].ap(), atb.rearrange("p a c -> p (a c)"))
                    # ---- stage E for this branch ----
                    for mt in range(2):
                        m0 = mt * 128
                        stg = st_pool.tile([128, 768], F32, tag="stg")
                        for (c0, cw) in ((0, 512), (512, 256)):
                            pr = psw.tile([128, 512], F32, tag="w2")
                            nc.tensor.matmul(pr[0:128, 0:cw], onesrow[0:1, :], biasr[0:1, c0:c0 + cw],
                                             start=True, stop=False)
                            for k in range(6):
                                nc.tensor.matmul(pr[0:128, 0:cw], atb[:, k, m0:m0 + 128], wp[:, k, c0:c0 + cw],
                                                 start=False, stop=(k == 5))
                            nc.vector.tensor_copy(stg[:, c0:c0 + cw], pr[0:128, 0:cw])
                        r0 = 1 + br * 256 + m0
                        nc.sync.dma_start(out_d[b].ap()[r0:r0 + 128, :], stg[:])

            run_batch(0)
            if not __import__("os").environ.get("KONLY0"):
                run_batch(1)

    nc.compile()
    _NC_CACHE["nc"] = nc
    return nc


def _prep_inputs(x, W_qkv, W_proj, b_proj):
    import ml_dtypes
    bf16 = ml_dtypes.bfloat16
    xt = np.ascontiguousarray(np.transpose(np.asarray(x, np.float32), (0, 2, 1))).astype(bf16)
    wq = np.asarray(W_qkv, np.float32).astype(bf16)
    wpj = np.asarray(W_proj, np.float32).astype(bf16)
    bias = np.ascontiguousarray(np.asarray(b_proj, np.float32).reshape(1, 768)).astype(bf16)
    return xt, wq, wpj, bias


def kernel(x, W_qkv, W_proj, b_proj):
    _ensure_ntff_hook()
    from concourse import bass_utils
    nc = build_program()
    xt, wq, wpj, bias = _prep_inputs(x, W_qkv, W_proj, b_proj)
    in_maps = [{"xt0": xt[2 * c], "xt1": xt[2 * c + 1],
                "wqkv": wq, "wproj": wpj, "bias": bias}
               for c in range(8)]
    res = bass_utils.run_bass_kernel_spmd(nc, in_maps, list(range(8)))
    out = np.empty((16, 1025, 768), np.float32)
    for c in range(8):
        out[2 * c] = res.results[c]["out0"]
        out[2 * c + 1] = res.results[c]["out1"]
    return out


# revision 20
# speedup vs baseline: 1.3588x; 1.0420x over previous
"""Trainium2 Bass kernel for nn_Attention_Sep (sparse attention, B=16 N=1025 C=768 H=12 nb=4).

v2: all-bf16 data path (host-cast inputs), persistent weights in SBUF, dense PE
scheduling with batch interleaving via disjoint per-batch buffers.

Data-parallel over batch (2 per core, 8 cores). Per core, transposed
[feature, token] layout:
  A) Q^T/K^T feature-tiles = W_qkv.T @ x^T (bf16); V natural [token, feature]
     stored in 132-wide head-pair groups [A(64)|onesA|B(64)|onesB] so the PV
     matmul emits softmax denominators on PSUM row 64 for both heads.
  B) cls attention over all 1025 tokens: block-diagonal q0 lhsT -> [12, tok]
     logits, row softmax on ACT (exp+accum), PE-transposed weights, V
     contraction, residual -> clsat.
  C) recompute k,v of the updated cls token (row matmuls + tiny transposes).
  D) cls-key logits hoisted per head-pair via block lhsT [kcA|0..|kcB] (out
     rows 0/64); per (branch, pair): S^T QK, exp (ACT, scale 1/8), PV with
     denominators at row 64; reciprocal_approx_fast + one broadcast DMA + DVE
     muls -> per-branch A^T tile.
  E) projection per branch (as soon as the branch finishes): bias folded into
     the matmul via a ones-row, PSUM->SBUF copy, DMA out.
"""
import sys, types
import numpy as np


def _ensure_ntff_hook():
    try:
        import antenv
        if "antenv.axon_hooks" in sys.modules:
            return
        from trn_agent_boot.trn_boot import _ntff_profile_via_ctypes
        mod = types.ModuleType("antenv.axon_hooks")
        mod._hook = None
        mod.set_axon_ntff_profile_hook = lambda h: setattr(mod, "_hook", h)
        mod.get_axon_ntff_profile_hook = lambda: mod._hook
        sys.modules["antenv.axon_hooks"] = mod
        antenv.axon_hooks = mod
        mod.set_axon_ntff_profile_hook(_ntff_profile_via_ctypes('/opt/axon/libaxon_pjrt.so'))
    except Exception:
        pass


_NC_CACHE = {}

G = 132          # head-pair group width in vstore: [A(0:64)|onesA(64)|B(65:129)|onesB(129)|pad]
ASL = (0, 65)    # A-slice of a group (A feats + onesA)
BSL = (65, 130)  # B-slice (B feats + onesB), feats->out rows 0:64, den->row 64


def build_program():
    if "nc" in _NC_CACHE:
        return _NC_CACHE["nc"]
    import concourse.bass as bass
    import concourse.mybir as mybir
    import concourse.tile as tile
    from concourse import bacc
    from concourse.masks import make_identity

    F32, BF16 = mybir.dt.float32, mybir.dt.bfloat16
    AF = mybir.ActivationFunctionType
    SCALE = 0.125

    nc = bacc.Bacc("TRN2", target_bir_lowering=False, debug=False)
    xt_d = [nc.dram_tensor(f"xt{b}", [768, 1025], BF16, kind="ExternalInput") for b in range(2)]
    wq_d = nc.dram_tensor("wqkv", [768, 2304], BF16, kind="ExternalInput")
    wp_d = nc.dram_tensor("wproj", [768, 768], BF16, kind="ExternalInput")
    bias_d = nc.dram_tensor("bias", [1, 768], BF16, kind="ExternalInput")
    out_d = [nc.dram_tensor(f"out{b}", [1025, 768], F32, kind="ExternalOutput") for b in range(2)]
    DEBUG = bool(__import__("os").environ.get("KDBG"))
    if DEBUG:
        dbg_qkt = [nc.dram_tensor(f"dqkt{b}", [128, 12 * 1024], mybir.dt.bfloat16, kind="ExternalOutput") for b in range(2)]
        dbg_cls = [nc.dram_tensor(f"dcls{b}", [128, 6], mybir.dt.bfloat16, kind="ExternalOutput") for b in range(2)]
        dbg_esc = [nc.dram_tensor(f"desc{b}", [2, 6 * 1024], mybir.dt.bfloat16, kind="ExternalOutput") for b in range(2)]
        dbg_kvc = [nc.dram_tensor(f"dkvc{b}", [1, 1536], mybir.dt.bfloat16, kind="ExternalOutput") for b in range(2)]
        dbg_atb = [nc.dram_tensor(f"datb{b}", [128, 6 * 256], mybir.dt.bfloat16, kind="ExternalOutput") for b in range(2)]
        dbg_vst = [nc.dram_tensor(f"dvst{b}", [128, 6 * 130], mybir.dt.bfloat16, kind="ExternalOutput") for b in range(2)]

    with tile.TileContext(nc) as tc:
        with (
            tc.tile_pool(name="big", bufs=1) as big,
            tc.tile_pool(name="pb", bufs=1) as pb,
            tc.tile_pool(name="xe", bufs=2) as xe,      # xt / esc alias ring
            tc.tile_pool(name="es", bufs=4) as es_pool,
            tc.tile_pool(name="rb", bufs=4) as rb_pool,
            tc.tile_pool(name="bb", bufs=3) as bb_pool,
            tc.tile_pool(name="ab2", bufs=3) as ab2_pool,
            tc.tile_pool(name="st", bufs=2) as st_pool,
            tc.tile_pool(name="ab", bufs=2) as ab_pool,
            tc.tile_pool(name="row", bufs=2) as row_pool,
            tc.tile_pool(name="pss", bufs=2, space="PSUM") as pss,   # [128,1024] S^T  -> 4 banks
            tc.tile_pool(name="pso", bufs=2, space="PSUM") as pso,   # [128,512]  PV   -> 2 banks
            tc.tile_pool(name="psw", bufs=2, space="PSUM") as psw,   # [128,512]  misc -> 2 banks
        ):
            # ---------- persistent weights ----------
            wq_r = wq_d.ap().rearrange("(k p) c -> p k c", p=128)
            wqk = big.tile([128, 6, 1536], BF16, tag="wqk")
            for half in range(2):
                nc.sync.dma_start(
                    wqk[:, :, half * 768:(half + 1) * 768],
                    wq_r[:, :, half * 768:(half + 1) * 768])
            wv = big.tile([128, 6, 768], BF16, tag="wv")
            nc.sync.dma_start(wv[:], wq_r[:, :, 1536:2304])
            wp = big.tile([128, 6, 768], BF16, tag="wp")
            nc.sync.dma_start(wp[:], wp_d.ap().rearrange("(k p) c -> p k c", p=128))
            biasr = big.tile([1, 768], BF16, tag="biasr")
            nc.sync.dma_start(biasr[:], bias_d.ap())
            ident = big.tile([128, 128], BF16, tag="ident")
            make_identity(nc, ident[:])
            onesrow = big.tile([1, 128], BF16, tag="onesrow")
            nc.vector.memset(onesrow[:], 1.0)

            # ---------- per-batch tiles ----------
            qkt = [pb.tile([128, 12, 1024], BF16, tag=f"qkt{b}", name=f"qkt{b}") for b in range(2)]
            vst = [pb.tile([128, 8, 6, G], BF16, tag=f"vst{b}", name=f"vst{b}") for b in range(2)]
            x0col = [pb.tile([128, 6], BF16, tag=f"x0c{b}", name=f"x0c{b}") for b in range(2)]
            q0cp = [pb.tile([128, 6], BF16, tag=f"q0c{b}", name=f"q0c{b}") for b in range(2)]
            k0cp = [pb.tile([128, 6], BF16, tag=f"k0c{b}", name=f"k0c{b}") for b in range(2)]
            kct = [pb.tile([128, 6], BF16, tag=f"kct{b}", name=f"kct{b}") for b in range(2)]
            clsat = [pb.tile([128, 6], BF16, tag=f"ca{b}", name=f"ca{b}") for b in range(2)]
            q0blk = [pb.tile([128, 6, 12], BF16, tag=f"q0b{b}", name=f"q0b{b}") for b in range(2)]
            kblk = [pb.tile([128, 6, 65], BF16, tag=f"kb{b}", name=f"kb{b}") for b in range(2)]
            wclsb = [pb.tile([12, 1025], BF16, tag=f"wc{b}", name=f"wc{b}") for b in range(2)]
            wtcls = [pb.tile([128, 9, 12], BF16, tag=f"wt{b}", name=f"wt{b}") for b in range(2)]
            part = [pb.tile([12, 4], F32, tag=f"pt{b}", name=f"pt{b}") for b in range(2)]
            v0a = [pb.tile([65, 6, G], BF16, tag=f"v0a{b}", name=f"v0a{b}") for b in range(2)]
            vca = [pb.tile([65, 6, G], BF16, tag=f"vca{b}", name=f"vca{b}") for b in range(2)]

            # ones columns of vstore groups (once per batch buffer)
            for b in range(2):
                nc.vector.memset(
                    bass.AP(vst[b].tensor, vst[b].offset + 64,
                            [[8 * 6 * G, 128], [6 * G, 8], [G, 6], [65, 2]]), 1.0)
                for t in (v0a[b], vca[b]):
                    nc.vector.memset(
                        bass.AP(t.tensor, t.offset + 64, [[6 * G, 1], [G, 6], [1, 1]]), 1.0)
                    nc.vector.memset(
                        bass.AP(t.tensor, t.offset + 64 * 6 * G + 129, [[6 * G, 1], [G, 6], [1, 1]]), 1.0)
                nc.vector.memset(q0blk[b][:], 0.0)
                nc.vector.memset(kblk[b][:], 0.0)

            VS_P = 8 * 6 * G

            def run_abc(b):
                # ================= stage A =================
                xt = xe.tile([128, 6, 1025], BF16, tag="xtesc")
                nc.sync.dma_start(xt[:], xt_d[b].ap().rearrange("(k p) n -> p k n", p=128))
                nc.vector.tensor_copy(
                    x0col[b][:],
                    bass.AP(xt.tensor, xt.offset, [[6 * 1025, 128], [1025, 6]]))
                # Q^T, K^T feature tiles
                for m in range(12):
                    for ci in range(2):
                        qk_ps = psw.tile([128, 512], F32, tag="w2")
                        for k in range(6):
                            nc.tensor.matmul(qk_ps[:], wqk[:, k, m * 128:(m + 1) * 128],
                                             xt[:, k, 1 + ci * 512: 1 + (ci + 1) * 512],
                                             start=(k == 0), stop=(k == 5))
                        if ci == 0:
                            nc.vector.tensor_copy(qkt[b][:, m, 0:512], qk_ps[:])
                        else:
                            nc.scalar.copy(qkt[b][:, m, 512:1024], qk_ps[:])
                # V natural into grouped layout
                for t in range(8):
                    for ci, (c0, cw, g0) in enumerate(((0, 512, 0), (512, 256, 4))):
                        v_ps = psw.tile([128, 512], F32, tag="w2")
                        for k in range(6):
                            nc.tensor.matmul(v_ps[:, 0:cw], xt[:, k, 1 + t * 128: 1 + (t + 1) * 128],
                                             wv[:, k, c0:c0 + cw], start=(k == 0), stop=(k == 5))
                        dst = bass.AP(vst[b].tensor, vst[b].offset + t * 6 * G + g0 * G,
                                      [[VS_P, 128], [G, cw // 128], [65, 2], [1, 64]])
                        nc.vector.tensor_copy(dst, v_ps[:, 0:cw].rearrange("p (g h d) -> p g h d", h=2, d=64))
                # q0|k0|v0 row of the original cls token
                qkv0 = row_pool.tile([1, 2304], BF16, tag="rowq")
                for (c0, cw) in ((0, 512), (512, 512), (1024, 512), (1536, 512), (2048, 256)):
                    r_ps = psw.tile([128, 512], F32, tag="w2")
                    for k in range(6):
                        rhs = wqk[:, k, c0:c0 + cw] if c0 < 1536 else wv[:, k, c0 - 1536:c0 - 1536 + cw]
                        nc.tensor.matmul(r_ps[0:1, 0:cw], x0col[b][:, k:k + 1], rhs,
                                         start=(k == 0), stop=(k == 5))
                    nc.vector.tensor_copy(qkv0[0:1, c0:c0 + cw], r_ps[0:1, 0:cw])
                for k in range(6):
                    tq = psw.tile([128, 512], BF16, tag="w2")
                    nc.tensor.transpose(tq[0:128, 0:1], qkv0[0:1, k * 128:(k + 1) * 128], ident[0:1, 0:1])
                    nc.vector.tensor_copy(q0cp[b][:, k:k + 1], tq[0:128, 0:1])
                    tk = psw.tile([128, 512], BF16, tag="w2")
                    nc.tensor.transpose(tk[0:128, 0:1], qkv0[0:1, 768 + k * 128: 768 + (k + 1) * 128],
                                        ident[0:1, 0:1])
                    nc.vector.tensor_copy(k0cp[b][:, k:k + 1], tk[0:128, 0:1])
                # v0 -> aug groups (A at row 0, B at row 64)
                nc.vector.tensor_copy(
                    bass.AP(v0a[b].tensor, v0a[b].offset, [[6 * G, 1], [G, 6], [1, 64]]),
                    bass.AP(qkv0.tensor, qkv0.offset + 1536, [[2304, 1], [128, 6], [1, 64]]))
                nc.vector.tensor_copy(
                    bass.AP(v0a[b].tensor, v0a[b].offset + 64 * 6 * G + 65, [[6 * G, 1], [G, 6], [1, 64]]),
                    bass.AP(qkv0.tensor, qkv0.offset + 1600, [[2304, 1], [128, 6], [1, 64]]))

                # ================= stage B: cls attention =================
                nc.vector.tensor_copy(
                    bass.AP(q0blk[b].tensor, q0blk[b].offset, [[6 * 12, 64], [14, 6]]),
                    bass.AP(q0cp[b].tensor, q0cp[b].offset, [[6, 64], [1, 6]]))
                nc.vector.tensor_copy(
                    bass.AP(q0blk[b].tensor, q0blk[b].offset + 64 * 6 * 12 + 1, [[6 * 12, 64], [14, 6]]),
                    bass.AP(q0cp[b].tensor, q0cp[b].offset + 64 * 6, [[6, 64], [1, 6]]))
                for ci in range(2):
                    cl_ps = psw.tile([128, 512], F32, tag="w2")
                    for k in range(6):
                        nc.tensor.matmul(cl_ps[0:12, :], q0blk[b][:, k], qkt[b][:, 6 + k, ci * 512:(ci + 1) * 512],
                                         start=(k == 0), stop=(k == 5))
                    nc.scalar.activation(wclsb[b][:, 1 + ci * 512: 1 + (ci + 1) * 512], cl_ps[0:12, :],
                                         AF.Exp, scale=SCALE, accum_out=part[b][:, ci:ci + 1])
                cl0 = psw.tile([128, 512], F32, tag="w2")
                for k in range(6):
                    nc.tensor.matmul(cl0[0:12, 0:1], q0blk[b][:, k], k0cp[b][:, k:k + 1],
                                     start=(k == 0), stop=(k == 5))
                nc.scalar.activation(wclsb[b][:, 0:1], cl0[0:12, 0:1], AF.Exp, scale=SCALE,
                                     accum_out=part[b][:, 2:3])
                nc.vector.tensor_add(part[b][:, 3:4], part[b][:, 0:1], part[b][:, 1:2])
                nc.vector.tensor_add(part[b][:, 3:4], part[b][:, 3:4], part[b][:, 2:3])
                nc.vector.reciprocal(part[b][:, 3:4], part[b][:, 3:4])
                nc.vector.tensor_scalar_mul(wclsb[b][:], wclsb[b][:], part[b][:, 3:4])
                t0 = psw.tile([128, 512], BF16, tag="w2")
                nc.tensor.transpose(t0[0:1, 0:12], wclsb[b][:, 0:1], ident[0:12, 0:12])
                nc.vector.tensor_copy(wtcls[b][0:1, 0, :], t0[0:1, 0:12])
                nc.vector.tensor_copy(wtcls[b][64:65, 0, :], t0[0:1, 0:12])
                for t in range(8):
                    tw = psw.tile([128, 512], BF16, tag="w2")
                    nc.tensor.transpose(tw[0:128, 0:12], wclsb[b][:, 1 + t * 128: 1 + (t + 1) * 128],
                                        ident[0:12, 0:12])
                    nc.vector.tensor_copy(wtcls[b][:, 1 + t, :], tw[0:128, 0:12])
                for fi in range(6):
                    co_a = pso.tile([128, 512], F32, tag="o")
                    co_b = pso.tile([128, 512], F32, tag="o")
                    lA0 = bass.AP(v0a[b].tensor, v0a[b].offset + fi * G, [[6 * G, 1], [1, 65]])
                    nc.tensor.matmul(co_a[0:65, 0:12], lA0, wtcls[b][0:1, 0, :], start=True, stop=False)
                    lB0 = bass.AP(v0a[b].tensor, v0a[b].offset + 64 * 6 * G + fi * G + 65, [[6 * G, 1], [1, 65]])
                    nc.tensor.matmul(co_b[0:65, 0:12], lB0, wtcls[b][64:65, 0, :], start=True, stop=False)
                    for t in range(8):
                        lA = bass.AP(vst[b].tensor, vst[b].offset + t * 6 * G + fi * G, [[VS_P, 128], [1, 65]])
                        nc.tensor.matmul(co_a[0:65, 0:12], lA, wtcls[b][:, 1 + t, :],
                                         start=False, stop=(t == 7))
                        lB = bass.AP(vst[b].tensor, vst[b].offset + t * 6 * G + fi * G + 65, [[VS_P, 128], [1, 65]])
                        nc.tensor.matmul(co_b[0:65, 0:12], lB, wtcls[b][:, 1 + t, :],
                                         start=False, stop=(t == 7))
                    nc.vector.tensor_add(clsat[b][0:64, fi:fi + 1], co_a[0:64, 2 * fi:2 * fi + 1],
                                         x0col[b][0:64, fi:fi + 1])
                    nc.vector.tensor_add(clsat[b][64:128, fi:fi + 1], co_b[0:64, 2 * fi + 1:2 * fi + 2],
                                         x0col[b][64:128, fi:fi + 1])

                # ---- cls output row (projection of clsat) ----
                stgc = st_pool.tile([128, 768], F32, tag="stg")
                for (c0, cw) in ((0, 512), (512, 256)):
                    pr = psw.tile([128, 512], F32, tag="w2")
                    nc.tensor.matmul(pr[0:1, 0:cw], onesrow[0:1, 0:1], biasr[0:1, c0:c0 + cw],
                                     start=True, stop=False)
                    for k in range(6):
                        nc.tensor.matmul(pr[0:1, 0:cw], clsat[b][:, k:k + 1], wp[:, k, c0:c0 + cw],
                                         start=False, stop=(k == 5))
                    nc.vector.tensor_copy(stgc[0:1, c0:c0 + cw], pr[0:1, 0:cw])
                nc.sync.dma_start(out_d[b].ap()[0:1, :], stgc[0:1, :])

                # ================= stage C: updated cls k, v =================
                kvc = row_pool.tile([1, 1536], BF16, tag="rowq")
                for (c0, cw) in ((0, 512), (512, 256), (768, 512), (1280, 256)):
                    kv_ps = psw.tile([128, 512], F32, tag="w2")
                    for k in range(6):
                        rhs = wqk[:, k, 768 + c0:768 + c0 + cw] if c0 < 768 else wv[:, k, c0 - 768:c0 - 768 + cw]
                        nc.tensor.matmul(kv_ps[0:1, 0:cw], clsat[b][:, k:k + 1], rhs,
                                         start=(k == 0), stop=(k == 5))
                    nc.vector.tensor_copy(kvc[0:1, c0:c0 + cw], kv_ps[0:1, 0:cw])
                for k in range(6):
                    tk = psw.tile([128, 512], BF16, tag="w2")
                    nc.tensor.transpose(tk[0:128, 0:1], kvc[0:1, k * 128:(k + 1) * 128], ident[0:1, 0:1])
                    nc.vector.tensor_copy(kct[b][:, k:k + 1], tk[0:128, 0:1])
                # kblk: [kcA | 0..0 | kcB] block columns (col 0 rows 0:64, col 64 rows 64:128)
                nc.vector.tensor_copy(
                    bass.AP(kblk[b].tensor, kblk[b].offset, [[6 * 65, 64], [65, 6]]),
                    bass.AP(kct[b].tensor, kct[b].offset, [[6, 64], [1, 6]]))
                nc.vector.tensor_copy(
                    bass.AP(kblk[b].tensor, kblk[b].offset + 64 * 6 * 65 + 64, [[6 * 65, 64], [65, 6]]),
                    bass.AP(kct[b].tensor, kct[b].offset + 64 * 6, [[6, 64], [1, 6]]))
                # vca aug groups
                nc.vector.tensor_copy(
                    bass.AP(vca[b].tensor, vca[b].offset, [[6 * G, 1], [G, 6], [1, 64]]),
                    bass.AP(kvc.tensor, kvc.offset + 768, [[1536, 1], [128, 6], [1, 64]]))
                nc.vector.tensor_copy(
                    bass.AP(vca[b].tensor, vca[b].offset + 64 * 6 * G + 65, [[6 * G, 1], [G, 6], [1, 64]]),
                    bass.AP(kvc.tensor, kvc.offset + 832, [[1536, 1], [128, 6], [1, 64]]))

            def run_d(b):
                # ================= stage D: branch attention =================
                # hoisted cls-key logits: esc rows 0 (head A) / 64 (head B)
                esc = xe.tile([128, 6, 1024], BF16, tag="xtesc")
                for fi in range(6):
                    cpa = pss.tile([128, 1024], F32, tag="s")
                    cpb = pss.tile([128, 1024], F32, tag="s")
                    for ci in range(2):
                        nc.tensor.matmul(cpa[0:1, ci * 512:(ci + 1) * 512], kblk[b][:, fi, 0:1],
                                         qkt[b][:, fi, ci * 512:(ci + 1) * 512], start=True, stop=True)
                        nc.tensor.matmul(cpb[64:65, ci * 512:(ci + 1) * 512], kblk[b][:, fi, 64:65],
                                         qkt[b][:, fi, ci * 512:(ci + 1) * 512], start=True, stop=True)
                    nc.scalar.activation(esc[0:1, fi, :], cpa[0:1, :], AF.Exp, scale=SCALE)
                    nc.scalar.activation(esc[64:65, fi, :], cpb[64:65, :], AF.Exp, scale=SCALE)
                if DEBUG:
                    nc.sync.dma_start(dbg_qkt[b].ap(), qkt[b].rearrange("p a c -> p (a c)"))
                    nc.sync.dma_start(dbg_cls[b].ap(), clsat[b][:])
                    nc.sync.dma_start(dbg_esc[b].ap()[0:1, :], esc[0:1].rearrange("p a c -> p (a c)"))
                    nc.sync.dma_start(dbg_esc[b].ap()[1:2, :], esc[64:65].rearrange("p a c -> p (a c)"))
                    nc.sync.dma_start(dbg_vst[b].ap(), bass.AP(vst[b].tensor, vst[b].offset, [[VS_P, 128], [G, 6], [1, 130]]))
                for br in range(4):
                    atb = ab_pool.tile([128, 6, 256], BF16, tag="atb")
                    for fi in range(6):
                        qsl = slice(br * 256, (br + 1) * 256)
                        ps_s = pss.tile([128, 1024], F32, tag="s")
                        for half in range(2):
                            ksl = slice(br * 256 + half * 128, br * 256 + (half + 1) * 128)
                            nc.tensor.matmul(ps_s[:, half * 256:(half + 1) * 256],
                                             qkt[b][0:64, 6 + fi, ksl], qkt[b][0:64, fi, qsl],
                                             start=True, stop=True)
                            nc.tensor.matmul(ps_s[:, 512 + half * 256: 512 + (half + 1) * 256],
                                             qkt[b][64:128, 6 + fi, ksl], qkt[b][64:128, fi, qsl],
                                             start=True, stop=True)
                        esa = es_pool.tile([128, 512], BF16, tag="es")
                        esb = es_pool.tile([128, 512], BF16, tag="es")
                        nc.scalar.activation(esa[:], ps_s[:, 0:512], AF.Exp, scale=SCALE)
                        nc.scalar.activation(esb[:], ps_s[:, 512:1024], AF.Exp, scale=SCALE)
                        ps_o = pso.tile([128, 512], F32, tag="o")
                        lA0 = bass.AP(vca[b].tensor, vca[b].offset + fi * G, [[6 * G, 1], [1, 65]])
                        nc.tensor.matmul(ps_o[0:65, 0:256], lA0, esc[0:1, fi, qsl], start=True, stop=False)
                        for half in range(2):
                            tt = 2 * br + half
                            lA = bass.AP(vst[b].tensor, vst[b].offset + tt * 6 * G + fi * G, [[VS_P, 128], [1, 65]])
                            nc.tensor.matmul(ps_o[0:65, 0:256], lA, esa[:, half * 256:(half + 1) * 256],
                                             start=False, stop=(half == 1))
                        lB0 = bass.AP(vca[b].tensor, vca[b].offset + 64 * 6 * G + fi * G + 65, [[6 * G, 1], [1, 65]])
                        nc.tensor.matmul(ps_o[0:65, 256:512], lB0, esc[64:65, fi, qsl], start=True, stop=False)
                        for half in range(2):
                            tt = 2 * br + half
                            lB = bass.AP(vst[b].tensor, vst[b].offset + tt * 6 * G + fi * G + 65, [[VS_P, 128], [1, 65]])
                            nc.tensor.matmul(ps_o[0:65, 256:512], lB, esb[:, half * 256:(half + 1) * 256],
                                             start=False, stop=(half == 1))
                        # decouple ps_o: two quick copies release the PSUM bank,
                        # normalization happens off the PE critical path
                        atu = ab2_pool.tile([65, 512], BF16, tag="atu")
                        nc.scalar.copy(atu[0:65, :], ps_o[0:65, :])
                        rb = rb_pool.tile([1, 512], F32, tag="rb")
                        nc.vector.tensor_copy(rb[0:1, :], ps_o[64:65, :])
                        nc.vector.reciprocal_approx_fast(rb[0:1, :], rb[0:1, :])
                        rbb = bb_pool.tile([64, 512], F32, tag="rbb")
                        nc.sync.dma_start(rbb[0:64, :], bass.AP(rb.tensor, rb.offset,
                                                                [[512, 1], [0, 64], [1, 512]]))
                        nc.vector.tensor_mul(atb[0:64, fi, :], atu[0:64, 0:256], rbb[0:64, 0:256])
                        nc.vector.tensor_mul(atb[64:128, fi, :], atu[0:64, 256:512], rbb[0:64, 256:512])
                    if DEBUG and br == 0:
                        nc.sync.dma_start(dbg_atb[b].ap(), atb.rearrange("p a c -> p (a c)"))
                    # ---- stage E for this branch ----
                    for mt in range(2):
                        m0 = mt * 128
                        stg = st_pool.tile([128, 768], F32, tag="stg")
                        for (c0, cw) in ((0, 512), (512, 256)):
                            pr = psw.tile([128, 512], F32, tag="w2")
                            nc.tensor.matmul(pr[0:128, 0:cw], onesrow[0:1, :], biasr[0:1, c0:c0 + cw],
                                             start=True, stop=False)
                            for k in range(6):
                                nc.tensor.matmul(pr[0:128, 0:cw], atb[:, k, m0:m0 + 128], wp[:, k, c0:c0 + cw],
                                                 start=False, stop=(k == 5))
                            nc.vector.tensor_copy(stg[:, c0:c0 + cw], pr[0:128, 0:cw])
                        r0 = 1 + br * 256 + m0
                        nc.sync.dma_start(out_d[b].ap()[r0:r0 + 128, :], stg[:])

            run_abc(0)
            if __import__("os").environ.get("KONLY0"):
                run_d(0)
            else:
                run_abc(1)
                run_d(0)
                run_d(1)

    nc.compile()
    _NC_CACHE["nc"] = nc
    return nc


def _prep_inputs(x, W_qkv, W_proj, b_proj):
    import ml_dtypes
    bf16 = ml_dtypes.bfloat16
    xt = np.ascontiguousarray(np.transpose(np.asarray(x, np.float32), (0, 2, 1))).astype(bf16)
    wq = np.asarray(W_qkv, np.float32).astype(bf16)
    wpj = np.asarray(W_proj, np.float32).astype(bf16)
    bias = np.ascontiguousarray(np.asarray(b_proj, np.float32).reshape(1, 768)).astype(bf16)
    return xt, wq, wpj, bias


def kernel(x, W_qkv, W_proj, b_proj):
    _ensure_ntff_hook()
    from concourse import bass_utils
    nc = build_program()
    xt, wq, wpj, bias = _prep_inputs(x, W_qkv, W_proj, b_proj)
    in_maps = [{"xt0": xt[2 * c], "xt1": xt[2 * c + 1],
                "wqkv": wq, "wproj": wpj, "bias": bias}
               for c in range(8)]
    res = bass_utils.run_bass_kernel_spmd(nc, in_maps, list(range(8)))
    out = np.empty((16, 1025, 768), np.float32)
    for c in range(8):
        out[2 * c] = res.results[c]["out0"]
        out[2 * c + 1] = res.results[c]["out1"]
    return out


# revision 24
# speedup vs baseline: 1.5546x; 1.1441x over previous
"""Trainium2 Bass kernel for nn_Attention_Sep (sparse attention, B=16 N=1025 C=768 H=12 nb=4).

v2: all-bf16 data path (host-cast inputs), persistent weights in SBUF, dense PE
scheduling with batch interleaving via disjoint per-batch buffers.

Data-parallel over batch (2 per core, 8 cores). Per core, transposed
[feature, token] layout:
  A) Q^T/K^T feature-tiles = W_qkv.T @ x^T (bf16); V natural [token, feature]
     stored in 132-wide head-pair groups [A(64)|onesA|B(64)|onesB] so the PV
     matmul emits softmax denominators on PSUM row 64 for both heads.
  B) cls attention over all 1025 tokens: block-diagonal q0 lhsT -> [12, tok]
     logits, row softmax on ACT (exp+accum), PE-transposed weights, V
     contraction, residual -> clsat.
  C) recompute k,v of the updated cls token (row matmuls + tiny transposes).
  D) cls-key logits hoisted per head-pair via block lhsT [kcA|0..|kcB] (out
     rows 0/64); per (branch, pair): S^T QK, exp (ACT, scale 1/8), PV with
     denominators at row 64; reciprocal_approx_fast + one broadcast DMA + DVE
     muls -> per-branch A^T tile.
  E) projection per branch (as soon as the branch finishes): bias folded into
     the matmul via a ones-row, PSUM->SBUF copy, DMA out.
"""
import sys, types
import numpy as np


def _ensure_ntff_hook():
    try:
        import antenv
        if "antenv.axon_hooks" in sys.modules:
            return
        from trn_agent_boot.trn_boot import _ntff_profile_via_ctypes
        mod = types.ModuleType("antenv.axon_hooks")
        mod._hook = None
        mod.set_axon_ntff_profile_hook = lambda h: setattr(mod, "_hook", h)
        mod.get_axon_ntff_profile_hook = lambda: mod._hook
        sys.modules["antenv.axon_hooks"] = mod
        antenv.axon_hooks = mod
        mod.set_axon_ntff_profile_hook(_ntff_profile_via_ctypes('/opt/axon/libaxon_pjrt.so'))
    except Exception:
        pass


_NC_CACHE = {}

G = 132          # head-pair group width in vstore: [A(0:64)|onesA(64)|B(65:129)|onesB(129)|pad]
ASL = (0, 65)    # A-slice of a group (A feats + onesA)
BSL = (65, 130)  # B-slice (B feats + onesB), feats->out rows 0:64, den->row 64


def build_program():
    if "nc" in _NC_CACHE:
        return _NC_CACHE["nc"]
    import concourse.bass as bass
    import concourse.mybir as mybir
    import concourse.tile as tile
    from concourse import bacc
    from concourse.masks import make_identity

    F32, BF16 = mybir.dt.float32, mybir.dt.bfloat16
    AF = mybir.ActivationFunctionType
    SCALE = 0.125

    nc = bacc.Bacc("TRN2", target_bir_lowering=False, debug=False)
    xt_d = [nc.dram_tensor(f"xt{b}", [768, 1025], BF16, kind="ExternalInput") for b in range(2)]
    wq_d = nc.dram_tensor("wqkv", [768, 2304], BF16, kind="ExternalInput")
    wp_d = nc.dram_tensor("wproj", [768, 768], BF16, kind="ExternalInput")
    bias_d = nc.dram_tensor("bias", [1, 768], BF16, kind="ExternalInput")
    out_d = [nc.dram_tensor(f"out{b}", [1025, 768], F32, kind="ExternalOutput") for b in range(2)]
    DEBUG = bool(__import__("os").environ.get("KDBG"))
    if DEBUG:
        dbg_qkt = [nc.dram_tensor(f"dqkt{b}", [128, 12 * 1024], mybir.dt.bfloat16, kind="ExternalOutput") for b in range(2)]
        dbg_cls = [nc.dram_tensor(f"dcls{b}", [128, 6], mybir.dt.bfloat16, kind="ExternalOutput") for b in range(2)]
        dbg_esc = [nc.dram_tensor(f"desc{b}", [2, 6 * 1024], mybir.dt.bfloat16, kind="ExternalOutput") for b in range(2)]
        dbg_kvc = [nc.dram_tensor(f"dkvc{b}", [1, 1536], mybir.dt.bfloat16, kind="ExternalOutput") for b in range(2)]
        dbg_atb = [nc.dram_tensor(f"datb{b}", [128, 6 * 256], mybir.dt.bfloat16, kind="ExternalOutput") for b in range(2)]
        dbg_vst = [nc.dram_tensor(f"dvst{b}", [128, 6 * 130], mybir.dt.bfloat16, kind="ExternalOutput") for b in range(2)]

    with tile.TileContext(nc) as tc:
        with (
            tc.tile_pool(name="big", bufs=1) as big,
            tc.tile_pool(name="pb", bufs=1) as pb,
            tc.tile_pool(name="xe", bufs=2) as xe,      # xt / esc alias ring
            tc.tile_pool(name="es", bufs=4) as es_pool,
            tc.tile_pool(name="rb", bufs=4) as rb_pool,
            tc.tile_pool(name="bb", bufs=3) as bb_pool,
            tc.tile_pool(name="ab2", bufs=3) as ab2_pool,
            tc.tile_pool(name="st", bufs=2) as st_pool,
            tc.tile_pool(name="ab", bufs=3) as ab_pool,
            tc.tile_pool(name="row", bufs=2) as row_pool,
            tc.tile_pool(name="pss", bufs=2, space="PSUM") as pss,   # [128,1024] S^T  -> 4 banks
            tc.tile_pool(name="pso", bufs=2, space="PSUM") as pso,   # [128,512]  PV   -> 2 banks
            tc.tile_pool(name="psw", bufs=2, space="PSUM") as psw,   # [128,512]  misc -> 2 banks
        ):
            # ---------- persistent weights ----------
            wq_r = wq_d.ap().rearrange("(k p) c -> p k c", p=128)
            wqk = big.tile([128, 6, 1536], BF16, tag="wqk")
            for c0 in (0, 256, 512, 768):
                cw = 256 if c0 < 768 else 768
                nc.sync.dma_start(wqk[:, :, c0:c0 + cw], wq_r[:, :, c0:c0 + cw])
            wv = big.tile([128, 6, 768], BF16, tag="wv")
            nc.sync.dma_start(wv[:], wq_r[:, :, 1536:2304])
            wp = big.tile([128, 6, 768], BF16, tag="wp")
            nc.sync.dma_start(wp[:], wp_d.ap().rearrange("(k p) c -> p k c", p=128))
            biasr = big.tile([1, 768], BF16, tag="biasr")
            nc.sync.dma_start(biasr[:], bias_d.ap())
            ident = big.tile([128, 128], BF16, tag="ident")
            make_identity(nc, ident[:])
            onesrow = big.tile([1, 128], BF16, tag="onesrow")
            nc.vector.memset(onesrow[:], 1.0)

            # ---------- per-batch tiles ----------
            qkt = [pb.tile([128, 12, 1024], BF16, tag=f"qkt{b}", name=f"qkt{b}") for b in range(2)]
            vst = [pb.tile([128, 8, 6, G], BF16, tag=f"vst{b}", name=f"vst{b}") for b in range(2)]
            x0col = [pb.tile([128, 6], BF16, tag=f"x0c{b}", name=f"x0c{b}") for b in range(2)]
            q0cp = [pb.tile([128, 6], BF16, tag=f"q0c{b}", name=f"q0c{b}") for b in range(2)]
            k0cp = [pb.tile([128, 6], BF16, tag=f"k0c{b}", name=f"k0c{b}") for b in range(2)]
            kct = [pb.tile([128, 6], BF16, tag=f"kct{b}", name=f"kct{b}") for b in range(2)]
            clsat = [pb.tile([128, 6], BF16, tag=f"ca{b}", name=f"ca{b}") for b in range(2)]
            q0blk = [pb.tile([128, 6, 12], BF16, tag=f"q0b{b}", name=f"q0b{b}") for b in range(2)]
            kblk = [pb.tile([128, 6, 65], BF16, tag=f"kb{b}", name=f"kb{b}") for b in range(2)]
            wclsb = [pb.tile([12, 1025], BF16, tag=f"wc{b}", name=f"wc{b}") for b in range(2)]
            wtcls = [pb.tile([128, 9, 12], BF16, tag=f"wt{b}", name=f"wt{b}") for b in range(2)]
            part = [pb.tile([12, 4], F32, tag=f"pt{b}", name=f"pt{b}") for b in range(2)]
            v0a = [pb.tile([65, 6, G], BF16, tag=f"v0a{b}", name=f"v0a{b}") for b in range(2)]
            vca = [pb.tile([65, 6, G], BF16, tag=f"vca{b}", name=f"vca{b}") for b in range(2)]

            # ones columns of vstore groups (once per batch buffer)
            for b in range(2):
                nc.vector.memset(
                    bass.AP(vst[b].tensor, vst[b].offset + 64,
                            [[8 * 6 * G, 128], [6 * G, 8], [G, 6], [65, 2]]), 1.0)
                for t in (v0a[b], vca[b]):
                    nc.vector.memset(
                        bass.AP(t.tensor, t.offset + 64, [[6 * G, 1], [G, 6], [1, 1]]), 1.0)
                    nc.vector.memset(
                        bass.AP(t.tensor, t.offset + 64 * 6 * G + 129, [[6 * G, 1], [G, 6], [1, 1]]), 1.0)
                nc.vector.memset(q0blk[b][:], 0.0)
                nc.vector.memset(kblk[b][:], 0.0)

            VS_P = 8 * 6 * G

            def run_abc(b):
                # ================= stage A =================
                xt = xe.tile([128, 6, 1025], BF16, tag="xtesc")
                nc.sync.dma_start(xt[:], xt_d[b].ap().rearrange("(k p) n -> p k n", p=128))
                nc.vector.tensor_copy(
                    x0col[b][:],
                    bass.AP(xt.tensor, xt.offset, [[6 * 1025, 128], [1025, 6]]))
                # Q^T, K^T feature tiles
                for m in range(12):
                    for ci in range(2):
                        qk_ps = psw.tile([128, 512], F32, tag="w2")
                        for k in range(6):
                            nc.tensor.matmul(qk_ps[:], wqk[:, k, m * 128:(m + 1) * 128],
                                             xt[:, k, 1 + ci * 512: 1 + (ci + 1) * 512],
                                             start=(k == 0), stop=(k == 5))
                        if ci == 0:
                            nc.vector.tensor_copy(qkt[b][:, m, 0:512], qk_ps[:])
                        else:
                            nc.scalar.copy(qkt[b][:, m, 512:1024], qk_ps[:])
                # V natural into grouped layout
                for t in range(8):
                    for ci, (c0, cw, g0) in enumerate(((0, 512, 0), (512, 256, 4))):
                        v_ps = psw.tile([128, 512], F32, tag="w2")
                        for k in range(6):
                            nc.tensor.matmul(v_ps[:, 0:cw], xt[:, k, 1 + t * 128: 1 + (t + 1) * 128],
                                             wv[:, k, c0:c0 + cw], start=(k == 0), stop=(k == 5))
                        dst = bass.AP(vst[b].tensor, vst[b].offset + t * 6 * G + g0 * G,
                                      [[VS_P, 128], [G, cw // 128], [65, 2], [1, 64]])
                        nc.vector.tensor_copy(dst, v_ps[:, 0:cw].rearrange("p (g h d) -> p g h d", h=2, d=64))
                # q0|k0|v0 row of the original cls token
                qkv0 = row_pool.tile([1, 2304], BF16, tag="rowq")
                for (c0, cw) in ((0, 512), (512, 512), (1024, 512), (1536, 512), (2048, 256)):
                    r_ps = psw.tile([128, 512], F32, tag="w2")
                    for k in range(6):
                        rhs = wqk[:, k, c0:c0 + cw] if c0 < 1536 else wv[:, k, c0 - 1536:c0 - 1536 + cw]
                        nc.tensor.matmul(r_ps[0:1, 0:cw], x0col[b][:, k:k + 1], rhs,
                                         start=(k == 0), stop=(k == 5))
                    nc.vector.tensor_copy(qkv0[0:1, c0:c0 + cw], r_ps[0:1, 0:cw])
                for k in range(6):
                    tq = psw.tile([128, 512], BF16, tag="w2")
                    nc.tensor.transpose(tq[0:128, 0:1], qkv0[0:1, k * 128:(k + 1) * 128], ident[0:1, 0:1])
                    nc.vector.tensor_copy(q0cp[b][:, k:k + 1], tq[0:128, 0:1])
                    tk = psw.tile([128, 512], BF16, tag="w2")
                    nc.tensor.transpose(tk[0:128, 0:1], qkv0[0:1, 768 + k * 128: 768 + (k + 1) * 128],
                                        ident[0:1, 0:1])
                    nc.vector.tensor_copy(k0cp[b][:, k:k + 1], tk[0:128, 0:1])
                # v0 -> aug groups (A at row 0, B at row 64)
                nc.vector.tensor_copy(
                    bass.AP(v0a[b].tensor, v0a[b].offset, [[6 * G, 1], [G, 6], [1, 64]]),
                    bass.AP(qkv0.tensor, qkv0.offset + 1536, [[2304, 1], [128, 6], [1, 64]]))
                nc.vector.tensor_copy(
                    bass.AP(v0a[b].tensor, v0a[b].offset + 64 * 6 * G + 65, [[6 * G, 1], [G, 6], [1, 64]]),
                    bass.AP(qkv0.tensor, qkv0.offset + 1600, [[2304, 1], [128, 6], [1, 64]]))

                # ================= stage B: cls attention =================
                nc.vector.tensor_copy(
                    bass.AP(q0blk[b].tensor, q0blk[b].offset, [[6 * 12, 64], [14, 6]]),
                    bass.AP(q0cp[b].tensor, q0cp[b].offset, [[6, 64], [1, 6]]))
                nc.vector.tensor_copy(
                    bass.AP(q0blk[b].tensor, q0blk[b].offset + 64 * 6 * 12 + 1, [[6 * 12, 64], [14, 6]]),
                    bass.AP(q0cp[b].tensor, q0cp[b].offset + 64 * 6, [[6, 64], [1, 6]]))
                for ci in range(2):
                    cl_ps = psw.tile([128, 512], F32, tag="w2")
                    for k in range(6):
                        nc.tensor.matmul(cl_ps[0:12, :], q0blk[b][:, k], qkt[b][:, 6 + k, ci * 512:(ci + 1) * 512],
                                         start=(k == 0), stop=(k == 5))
                    nc.scalar.activation(wclsb[b][:, 1 + ci * 512: 1 + (ci + 1) * 512], cl_ps[0:12, :],
                                         AF.Exp, scale=SCALE, accum_out=part[b][:, ci:ci + 1])
                cl0 = psw.tile([128, 512], F32, tag="w2")
                for k in range(6):
                    nc.tensor.matmul(cl0[0:12, 0:1], q0blk[b][:, k], k0cp[b][:, k:k + 1],
                                     start=(k == 0), stop=(k == 5))
                nc.scalar.activation(wclsb[b][:, 0:1], cl0[0:12, 0:1], AF.Exp, scale=SCALE,
                                     accum_out=part[b][:, 2:3])
                nc.vector.tensor_add(part[b][:, 3:4], part[b][:, 0:1], part[b][:, 1:2])
                nc.vector.tensor_add(part[b][:, 3:4], part[b][:, 3:4], part[b][:, 2:3])
                nc.vector.reciprocal(part[b][:, 3:4], part[b][:, 3:4])
                nc.vector.tensor_scalar_mul(wclsb[b][:], wclsb[b][:], part[b][:, 3:4])
                t0 = psw.tile([128, 512], BF16, tag="w2")
                nc.tensor.transpose(t0[0:1, 0:12], wclsb[b][:, 0:1], ident[0:12, 0:12])
                nc.vector.tensor_copy(wtcls[b][0:1, 0, :], t0[0:1, 0:12])
                nc.vector.tensor_copy(wtcls[b][64:65, 0, :], t0[0:1, 0:12])
                for t in range(8):
                    tw = psw.tile([128, 512], BF16, tag="w2")
                    nc.tensor.transpose(tw[0:128, 0:12], wclsb[b][:, 1 + t * 128: 1 + (t + 1) * 128],
                                        ident[0:12, 0:12])
                    nc.vector.tensor_copy(wtcls[b][:, 1 + t, :], tw[0:128, 0:12])
                for fi in range(6):
                    co_a = pso.tile([128, 512], F32, tag="o")
                    co_b = pso.tile([128, 512], F32, tag="o")
                    lA0 = bass.AP(v0a[b].tensor, v0a[b].offset + fi * G, [[6 * G, 1], [1, 65]])
                    nc.tensor.matmul(co_a[0:65, 0:12], lA0, wtcls[b][0:1, 0, :], start=True, stop=False)
                    lB0 = bass.AP(v0a[b].tensor, v0a[b].offset + 64 * 6 * G + fi * G + 65, [[6 * G, 1], [1, 65]])
                    nc.tensor.matmul(co_b[0:65, 0:12], lB0, wtcls[b][64:65, 0, :], start=True, stop=False)
                    for t in range(8):
                        lA = bass.AP(vst[b].tensor, vst[b].offset + t * 6 * G + fi * G, [[VS_P, 128], [1, 65]])
                        nc.tensor.matmul(co_a[0:65, 0:12], lA, wtcls[b][:, 1 + t, :],
                                         start=False, stop=(t == 7))
                        lB = bass.AP(vst[b].tensor, vst[b].offset + t * 6 * G + fi * G + 65, [[VS_P, 128], [1, 65]])
                        nc.tensor.matmul(co_b[0:65, 0:12], lB, wtcls[b][:, 1 + t, :],
                                         start=False, stop=(t == 7))
                    nc.vector.tensor_add(clsat[b][0:64, fi:fi + 1], co_a[0:64, 2 * fi:2 * fi + 1],
                                         x0col[b][0:64, fi:fi + 1])
                    nc.vector.tensor_add(clsat[b][64:128, fi:fi + 1], co_b[0:64, 2 * fi + 1:2 * fi + 2],
                                         x0col[b][64:128, fi:fi + 1])

                # ---- cls output row (projection of clsat) ----
                stgc = st_pool.tile([128, 768], F32, tag="stg")
                for (c0, cw) in ((0, 512), (512, 256)):
                    pr = psw.tile([128, 512], F32, tag="w2")
                    nc.tensor.matmul(pr[0:1, 0:cw], onesrow[0:1, 0:1], biasr[0:1, c0:c0 + cw],
                                     start=True, stop=False)
                    for k in range(6):
                        nc.tensor.matmul(pr[0:1, 0:cw], clsat[b][:, k:k + 1], wp[:, k, c0:c0 + cw],
                                         start=False, stop=(k == 5))
                    nc.vector.tensor_copy(stgc[0:1, c0:c0 + cw], pr[0:1, 0:cw])
                nc.sync.dma_start(out_d[b].ap()[0:1, :], stgc[0:1, :])

                # ================= stage C: updated cls k, v =================
                kvc = row_pool.tile([1, 1536], BF16, tag="rowq")
                for (c0, cw) in ((0, 512), (512, 256), (768, 512), (1280, 256)):
                    kv_ps = psw.tile([128, 512], F32, tag="w2")
                    for k in range(6):
                        rhs = wqk[:, k, 768 + c0:768 + c0 + cw] if c0 < 768 else wv[:, k, c0 - 768:c0 - 768 + cw]
                        nc.tensor.matmul(kv_ps[0:1, 0:cw], clsat[b][:, k:k + 1], rhs,
                                         start=(k == 0), stop=(k == 5))
                    nc.vector.tensor_copy(kvc[0:1, c0:c0 + cw], kv_ps[0:1, 0:cw])
                for k in range(6):
                    tk = psw.tile([128, 512], BF16, tag="w2")
                    nc.tensor.transpose(tk[0:128, 0:1], kvc[0:1, k * 128:(k + 1) * 128], ident[0:1, 0:1])
                    nc.vector.tensor_copy(kct[b][:, k:k + 1], tk[0:128, 0:1])
                # kblk: [kcA | 0..0 | kcB] block columns (col 0 rows 0:64, col 64 rows 64:128)
                nc.vector.tensor_copy(
                    bass.AP(kblk[b].tensor, kblk[b].offset, [[6 * 65, 64], [65, 6]]),
                    bass.AP(kct[b].tensor, kct[b].offset, [[6, 64], [1, 6]]))
                nc.vector.tensor_copy(
                    bass.AP(kblk[b].tensor, kblk[b].offset + 64 * 6 * 65 + 64, [[6 * 65, 64], [65, 6]]),
                    bass.AP(kct[b].tensor, kct[b].offset + 64 * 6, [[6, 64], [1, 6]]))
                # vca aug groups
                nc.vector.tensor_copy(
                    bass.AP(vca[b].tensor, vca[b].offset, [[6 * G, 1], [G, 6], [1, 64]]),
                    bass.AP(kvc.tensor, kvc.offset + 768, [[1536, 1], [128, 6], [1, 64]]))
                nc.vector.tensor_copy(
                    bass.AP(vca[b].tensor, vca[b].offset + 64 * 6 * G + 65, [[6 * G, 1], [G, 6], [1, 64]]),
                    bass.AP(kvc.tensor, kvc.offset + 832, [[1536, 1], [128, 6], [1, 64]]))

            def run_esc(b):
                # hoisted cls-key logits: esc rows 0 (head A) / 64 (head B)
                esc = xe.tile([128, 6, 1024], BF16, tag="xtesc")
                for fi in range(6):
                    cpa = pss.tile([128, 1024], F32, tag="s")
                    cpb = pss.tile([128, 1024], F32, tag="s")
                    for ci in range(2):
                        nc.tensor.matmul(cpa[0:1, ci * 512:(ci + 1) * 512], kblk[b][:, fi, 0:1],
                                         qkt[b][:, fi, ci * 512:(ci + 1) * 512], start=True, stop=True)
                        nc.tensor.matmul(cpb[64:65, ci * 512:(ci + 1) * 512], kblk[b][:, fi, 64:65],
                                         qkt[b][:, fi, ci * 512:(ci + 1) * 512], start=True, stop=True)
                    nc.scalar.activation(esc[0:1, fi, :], cpa[0:1, :], AF.Exp, scale=SCALE)
                    nc.scalar.activation(esc[64:65, fi, :], cpb[64:65, :], AF.Exp, scale=SCALE)
                if DEBUG:
                    nc.sync.dma_start(dbg_qkt[b].ap(), qkt[b].rearrange("p a c -> p (a c)"))
                    nc.sync.dma_start(dbg_cls[b].ap(), clsat[b][:])
                    nc.sync.dma_start(dbg_esc[b].ap()[0:1, :], esc[0:1].rearrange("p a c -> p (a c)"))
                    nc.sync.dma_start(dbg_esc[b].ap()[1:2, :], esc[64:65].rearrange("p a c -> p (a c)"))
                    nc.sync.dma_start(dbg_vst[b].ap(), bass.AP(vst[b].tensor, vst[b].offset, [[VS_P, 128], [G, 6], [1, 130]]))
                return esc

            def run_d_branch(b, esc, br):
                    atb = ab_pool.tile([128, 6, 256], BF16, tag="atb")
                    for fi in range(6):
                        qsl = slice(br * 256, (br + 1) * 256)
                        ps_s = pss.tile([128, 1024], F32, tag="s")
                        for half in range(2):
                            ksl = slice(br * 256 + half * 128, br * 256 + (half + 1) * 128)
                            nc.tensor.matmul(ps_s[:, half * 256:(half + 1) * 256],
                                             qkt[b][0:64, 6 + fi, ksl], qkt[b][0:64, fi, qsl],
                                             start=True, stop=True)
                            nc.tensor.matmul(ps_s[:, 512 + half * 256: 512 + (half + 1) * 256],
                                             qkt[b][64:128, 6 + fi, ksl], qkt[b][64:128, fi, qsl],
                                             start=True, stop=True)
                        esa = es_pool.tile([128, 512], BF16, tag="es")
                        esb = es_pool.tile([128, 512], BF16, tag="es")
                        nc.scalar.activation(esa[:], ps_s[:, 0:512], AF.Exp, scale=SCALE)
                        nc.scalar.activation(esb[:], ps_s[:, 512:1024], AF.Exp, scale=SCALE)
                        ps_o = pso.tile([128, 512], F32, tag="o")
                        lA0 = bass.AP(vca[b].tensor, vca[b].offset + fi * G, [[6 * G, 1], [1, 65]])
                        nc.tensor.matmul(ps_o[0:65, 0:256], lA0, esc[0:1, fi, qsl], start=True, stop=False)
                        for half in range(2):
                            tt = 2 * br + half
                            lA = bass.AP(vst[b].tensor, vst[b].offset + tt * 6 * G + fi * G, [[VS_P, 128], [1, 65]])
                            nc.tensor.matmul(ps_o[0:65, 0:256], lA, esa[:, half * 256:(half + 1) * 256],
                                             start=False, stop=(half == 1))
                        lB0 = bass.AP(vca[b].tensor, vca[b].offset + 64 * 6 * G + fi * G + 65, [[6 * G, 1], [1, 65]])
                        nc.tensor.matmul(ps_o[0:65, 256:512], lB0, esc[64:65, fi, qsl], start=True, stop=False)
                        for half in range(2):
                            tt = 2 * br + half
                            lB = bass.AP(vst[b].tensor, vst[b].offset + tt * 6 * G + fi * G + 65, [[VS_P, 128], [1, 65]])
                            nc.tensor.matmul(ps_o[0:65, 256:512], lB, esb[:, half * 256:(half + 1) * 256],
                                             start=False, stop=(half == 1))
                        # decouple ps_o: two quick copies release the PSUM bank,
                        # normalization happens off the PE critical path
                        atu = ab2_pool.tile([65, 512], BF16, tag="atu")
                        nc.scalar.copy(atu[0:65, :], ps_o[0:65, :])
                        rb = rb_pool.tile([1, 512], F32, tag="rb")
                        nc.vector.tensor_copy(rb[0:1, :], ps_o[64:65, :])
                        nc.vector.reciprocal_approx_fast(rb[0:1, :], rb[0:1, :])
                        rbb = bb_pool.tile([64, 512], F32, tag="rbb")
                        nc.sync.dma_start(rbb[0:64, :], bass.AP(rb.tensor, rb.offset,
                                                                [[512, 1], [0, 64], [1, 512]]))
                        nc.vector.tensor_mul(atb[0:64, fi, :], atu[0:64, 0:256], rbb[0:64, 0:256])
                        nc.vector.tensor_mul(atb[64:128, fi, :], atu[0:64, 256:512], rbb[0:64, 256:512])
                    if DEBUG and br == 0:
                        nc.sync.dma_start(dbg_atb[b].ap(), atb.rearrange("p a c -> p (a c)"))
                    return atb

            def run_e_branch(b, atb, br):
                    for mt in range(2):
                        m0 = mt * 128
                        stg = st_pool.tile([128, 768], F32, tag="stg")
                        for (c0, cw) in ((0, 512), (512, 256)):
                            pr = psw.tile([128, 512], F32, tag="w2")
                            nc.tensor.matmul(pr[0:128, 0:cw], onesrow[0:1, :], biasr[0:1, c0:c0 + cw],
                                             start=True, stop=False)
                            for k in range(6):
                                nc.tensor.matmul(pr[0:128, 0:cw], atb[:, k, m0:m0 + 128], wp[:, k, c0:c0 + cw],
                                                 start=False, stop=(k == 5))
                            nc.vector.tensor_copy(stg[:, c0:c0 + cw], pr[0:128, 0:cw])
                        r0 = 1 + br * 256 + m0
                        nc.sync.dma_start(out_d[b].ap()[r0:r0 + 128, :], stg[:])

            run_abc(0)
            if __import__("os").environ.get("KONLY0"):
                esc0 = run_esc(0)
                for br in range(4):
                    atb = run_d_branch(0, esc0, br)
                    run_e_branch(0, atb, br)
            else:
                run_abc(1)
                escs = [run_esc(0), run_esc(1)]
                pend = []
                for br in range(4):
                    for b in range(2):
                        atb = run_d_branch(b, escs[b], br)
                        pend.append((b, atb, br))
                        if len(pend) > 1:
                            run_e_branch(*pend.pop(0))
                for args in pend:
                    run_e_branch(*args)

    nc.compile()
    _NC_CACHE["nc"] = nc
    return nc


def _prep_inputs(x, W_qkv, W_proj, b_proj):
    import ml_dtypes
    bf16 = ml_dtypes.bfloat16
    xt = np.ascontiguousarray(np.transpose(np.asarray(x, np.float32), (0, 2, 1))).astype(bf16)
    wq = np.asarray(W_qkv, np.float32).astype(bf16)
    wpj = np.asarray(W_proj, np.float32).astype(bf16)
    bias = np.ascontiguousarray(np.asarray(b_proj, np.float32).reshape(1, 768)).astype(bf16)
    return xt, wq, wpj, bias


def kernel(x, W_qkv, W_proj, b_proj):
    _ensure_ntff_hook()
    from concourse import bass_utils
    nc = build_program()
    xt, wq, wpj, bias = _prep_inputs(x, W_qkv, W_proj, b_proj)
    in_maps = [{"xt0": xt[2 * c], "xt1": xt[2 * c + 1],
                "wqkv": wq, "wproj": wpj, "bias": bias}
               for c in range(8)]
    res = bass_utils.run_bass_kernel_spmd(nc, in_maps, list(range(8)))
    out = np.empty((16, 1025, 768), np.float32)
    for c in range(8):
        out[2 * c] = res.results[c]["out0"]
        out[2 * c + 1] = res.results[c]["out1"]
    return out
